# revision 14
# baseline (speedup 1.0000x reference)
"""Multi-head self-attention 2d (B=2, C=256, H=W=64, 8 heads x 32 dim) on 8 TRN2 cores.

Sharding: batch (2-way) x query-rows-of-N=H*W (4-way) => 8 cores, no collectives.
Each core computes, for its (batch b, query shard of 1024 rows):
  - K', V projections over the FULL 4096 positions of its batch (4x replicated work,
    but removes all cross-core communication),
  - Q' projection for its 1024 query rows,
  - S^T = K'^T-layout scores via tensor-engine row-packing (contraction d=32, 4 heads
    concurrently in the 128x128 PE array),
  - softmax without max-subtraction (scores are ~N(0,1); exp on ScalarE, denominator
    via ones-matmul, division on VectorE),
  - attn @ V via col-packed matmuls accumulating over the 4096 positions,
  - output projection, then out = gamma * proj + x on its [256, 1024] slice.
"""

import os
import sys

import numpy as np

for _p in ("/opt/trn_rl_repo", "/root/.axon_site/_ro/trn_rl_repo"):
    if os.path.isdir(_p) and _p not in sys.path:
        sys.path.insert(0, _p)

import ml_dtypes
import concourse.bacc as bacc
import concourse.bass as bass
import concourse.tile as tile
from concourse import mybir
from concourse.bass_utils import run_bass_kernel_spmd

BF16 = mybir.dt.bfloat16
F32 = mybir.dt.float32
NPBF16 = ml_dtypes.bfloat16

NH, D = 8, 32          # heads, head dim
C = 256                # channels
N = 4096               # H*W positions
Q = 1024               # query shard per core
SCALE = 1.0 / np.sqrt(D)

# Schraudolph bf16 exp on the vector engine: bf16_bits(exp(y)) ~= round(y*128/ln2
# + (127*128 - c)). Fold the attention scale into the multiplier. c tuned for
# min max-rel-error under round-to-nearest (~3.3%).
SCH_A = float(SCALE * 128.0 / np.log(2.0))
SCH_B = float(127.0 * 128.0 - 5.5)
# Fraction of exp tiles computed on ScalarE (rest on VectorE via Schraudolph),
# chosen to balance the two engines' total busy time (measured: ACT 1115ns,
# TS 1222ns per [128,1024] tile, plus ~90ns/m of other VectorE work).
ACT_UNITS_PER_32 = 17


def _build_program():
    nc = bacc.Bacc("TRN2", target_bir_lowering=False, debug=False)

    xb = nc.dram_tensor("xb", [C, N], BF16, kind="ExternalInput")
    xqb = nc.dram_tensor("xqb", [C, Q], BF16, kind="ExternalInput")
    xq = nc.dram_tensor("xq", [C, Q], F32, kind="ExternalInput")
    wkT = nc.dram_tensor("wkT", [C, C], BF16, kind="ExternalInput")
    wqT = nc.dram_tensor("wqT", [C, C], BF16, kind="ExternalInput")
    wvT = nc.dram_tensor("wvT", [C, C], BF16, kind="ExternalInput")
    pjT = nc.dram_tensor("pjT", [C, C], BF16, kind="ExternalInput")
    gam = nc.dram_tensor("gam", [128, 1], F32, kind="ExternalInput")
    out = nc.dram_tensor("out", [C, Q], F32, kind="ExternalOutput")

    with tile.TileContext(nc) as tc:
        _emit(tc, xb, xqb, xq, wkT, wqT, wvT, pjT, gam, out)
    nc.compile()
    return nc


def _emit(tc, xb, xqb, xq, wkT, wqT, wvT, pjT, gam, out):
    from contextlib import ExitStack

    nc = tc.nc
    Exp = mybir.ActivationFunctionType.Exp

    with ExitStack() as ctx:
        per = ctx.enter_context(tc.tile_pool(name="persist", bufs=1))

        def ptile(name, shape, dtype):
            return per.tile(shape, dtype, name=name, tag=name)

        XB = [ptile(f"XB{i}", [128, N], BF16) for i in range(2)]
        XQB = [ptile(f"XQB{i}", [128, Q], BF16) for i in range(2)]
        XQ = [ptile(f"XQ{i}", [128, Q], F32) for i in range(2)]
        WK = [ptile(f"WK{i}", [128, C], BF16) for i in range(2)]
        WQ = [ptile(f"WQ{i}", [128, C], BF16) for i in range(2)]
        WV = [ptile(f"WV{i}", [128, C], BF16) for i in range(2)]
        PJ = [ptile(f"PJ{i}", [128, C], BF16) for i in range(2)]
        G = ptile("G", [128, 1], F32)
        ONE = ptile("ONE", [128, 1], BF16)
        Z128 = ptile("Z128", [1, 128], BF16)   # zeros, lhsT of bank-zeroing matmul
        ONES512 = ptile("ONES512", [1, 512], BF16)
        ONES32F = ptile("ONES32F", [1, 32], F32)  # lhsT of denominator-broadcast matmul
        Ksb = [ptile(f"Ksb{i}", [128, N], BF16) for i in range(2)]
        Qsb = [ptile(f"Qsb{i}", [128, Q], BF16) for i in range(2)]
        Vsb = [ptile(f"Vsb{m}", [128, NH * 33], BF16) for m in range(32)]
        Osb = [ptile(f"Osb{i}", [128, Q], BF16) for i in range(2)]

        # small critical tensors first so Q'/K' projections start ASAP
        for i in range(2):
            r = slice(128 * i, 128 * (i + 1))
            nc.sync.dma_start(WQ[i][:], wqT[r, :])
            nc.sync.dma_start(XQB[i][:], xqb[r, :])
            nc.sync.dma_start(WK[i][:], wkT[r, :])
            nc.sync.dma_start(WV[i][:], wvT[r, :])
            nc.sync.dma_start(PJ[i][:], pjT[r, :])
        nc.sync.dma_start(G[:], gam[:, :])
        for ch in range(4):  # chunked so projections can start early
            cs_ = slice(1024 * ch, 1024 * (ch + 1))
            for i in range(2):
                r = slice(128 * i, 128 * (i + 1))
                nc.sync.dma_start(XB[i][:, cs_], xb[r, cs_])
        for i in range(2):
            r = slice(128 * i, 128 * (i + 1))
            nc.sync.dma_start(XQ[i][:], xq[r, :])  # only needed at the end
        nc.vector.memset(ONE[:], 1.0)
        nc.vector.memset(Z128[:], 0.0)
        nc.vector.memset(ONES512[:], 1.0)
        nc.vector.memset(ONES32F[:], 1.0)

        # ---- projections: Q'[hd, q], K'[hd, m], V[m, hd] --------------------
        # PSUM->SBUF copies alternate ScalarE/VectorE so neither engine gates
        # the projection phase on its own.
        with tc.tile_pool(name="pp", bufs=4, space="PSUM") as pp:
            ncopy = 0

            def pcopy(dst, src):
                nonlocal ncopy
                if ncopy % 2 == 0:
                    nc.vector.tensor_copy(dst, src)
                else:
                    nc.scalar.copy(dst, src)
                ncopy += 1

            for p in range(2):
                hs = slice(128 * p, 128 * (p + 1))
                for t in range(2):
                    qp = pp.tile([128, 512], F32, name="qp", tag="pp")
                    ts_ = slice(512 * t, 512 * (t + 1))
                    nc.tensor.matmul(qp[:], lhsT=WQ[0][:, hs], rhs=XQB[0][:, ts_],
                                     start=True, stop=False)
                    nc.tensor.matmul(qp[:], lhsT=WQ[1][:, hs], rhs=XQB[1][:, ts_],
                                     start=False, stop=True)
                    pcopy(Qsb[p][:, ts_], qp[:])
            for t in range(8):
                ts_ = slice(512 * t, 512 * (t + 1))
                for p in range(2):
                    hs = slice(128 * p, 128 * (p + 1))
                    kp = pp.tile([128, 512], F32, name="kp", tag="pp")
                    nc.tensor.matmul(kp[:], lhsT=WK[0][:, hs], rhs=XB[0][:, ts_],
                                     start=True, stop=False)
                    nc.tensor.matmul(kp[:], lhsT=WK[1][:, hs], rhs=XB[1][:, ts_],
                                     start=False, stop=True)
                    pcopy(Ksb[p][:, ts_], kp[:])
            for m in range(32):
                vp = pp.tile([128, 512], F32, name="vp", tag="pp")
                ms = slice(128 * m, 128 * (m + 1))
                nc.tensor.matmul(vp[:, :C], lhsT=XB[0][:, ms], rhs=WV[0][:],
                                 start=True, stop=False)
                nc.tensor.matmul(vp[:, :C], lhsT=XB[1][:, ms], rhs=WV[1][:],
                                 start=False, stop=True)
                v3 = Vsb[m].rearrange("p (h w) -> p h w", w=33)
                pcopy(v3[:, :, 0:32],
                      vp[:, :C].rearrange("p (h w) -> p h w", w=32))
                nc.vector.memset(v3[:, :, 32:33], 1.0)

        # ---- attention ------------------------------------------------------
        with ExitStack() as actx:
            sp = actx.enter_context(tc.tile_pool(name="sp", bufs=3, space="PSUM"))
            opl = actx.enter_context(tc.tile_pool(name="opl", bufs=1, space="PSUM"))
            pb = actx.enter_context(tc.tile_pool(name="pb", bufs=4))
            db = actx.enter_context(tc.tile_pool(name="db", bufs=8))
            rb = actx.enter_context(tc.tile_pool(name="rb", bufs=2))
            ob = actx.enter_context(tc.tile_pool(name="ob", bufs=4))

            def emit_oproj(qh):
                # output projection + residual for one q-half; borrows an sp
                # score slot for PSUM so it can overlap a running phase
                qs = slice(512 * qh, 512 * (qh + 1))
                pjp = sp.tile([128, 1024], F32, name="pjp", tag="st2")
                for ct in range(2):
                    cs = slice(128 * ct, 128 * (ct + 1))
                    pp2 = pjp[:, 512 * ct:512 * (ct + 1)]
                    nc.tensor.matmul(pp2, lhsT=PJ[0][:, cs], rhs=Osb[0][:, qs],
                                     start=True, stop=False,
                                     skip_group_check=True)
                    nc.tensor.matmul(pp2, lhsT=PJ[1][:, cs], rhs=Osb[1][:, qs],
                                     start=False, stop=True,
                                     skip_group_check=True)
                    obt = ob.tile([128, 512], F32, name="obt", tag="obt")
                    nc.vector.scalar_tensor_tensor(
                        obt[:], pp2, G[:], XQ[ct][:, qs],
                        mybir.AluOpType.mult, mybir.AluOpType.add)
                    nc.sync.dma_start(out[cs, qs], obt[:])

            # qh outer so the qh=0 output projection can overlap the qh=1
            # phases; hg inner.
            def run_phase(hg, qh, prev_tail=None, inject=None, inject_m=3):
                    qs = slice(512 * qh, 512 * (qh + 1))
                    Op = [opl.tile([128, 512], F32, name=f"Op{j}", tag=f"Op{j}")
                          for j in range(2)]

                    def emit_zero():
                        # hardware start=True only zeroes the region an MM
                        # writes; partial-width accumulation chains must land
                        # on explicitly zeroed banks (stale PSUM junk leaks
                        # in otherwise)
                        for j in range(2):
                            nc.tensor.matmul(Op[j][:], lhsT=Z128[:],
                                             rhs=ONES512[:], start=True,
                                             stop=True, skip_group_check=True)
                    # software-pipelined at tile-pair granularity: the PE queue
                    # is strictly in-order, so each entry is emitted right
                    # after the entry whose completion satisfies its
                    # dependency: s(m+1,g) reuses the PSUM buffer freed by
                    # exp(m,g'); av(m,j) consumes exp(m,j)'s output.
                    pts_by_m = {}

                    def emit_s_pair(m, g):
                        ms = slice(128 * m, 128 * (m + 1))
                        st2 = sp.tile([128, 1024], F32, name="st2", tag="st2")
                        for j in range(2):
                            a = 2 * g + j
                            hh = slice(32 * a, 32 * (a + 1))
                            nc.tensor.matmul(
                                st2[:, 512 * j:512 * (j + 1)],
                                lhsT=Ksb[hg][hh, ms], rhs=Qsb[hg][hh, qs],
                                start=True, stop=True,
                                tile_position=(32 * a, 0))
                        pt2 = pb.tile([128, 1024], BF16, name="pt2", tag="pt2")
                        pts_by_m.setdefault(m, {})[g] = pt2
                        # last 3 m-tiles -> ScalarE (frees VectorE for the
                        # denominator/normalize chain at the phase boundary);
                        # first 2 m-tiles -> VectorE (ScalarE drains backlog)
                        k = 2 * m + g
                        if m >= 29:
                            on_scalar = True
                        elif m < 2:
                            on_scalar = False
                        else:
                            on_scalar = (k * ACT_UNITS_PER_32) % 32 \
                                < ACT_UNITS_PER_32
                        if on_scalar:
                            nc.scalar.activation(pt2[:], st2[:], Exp,
                                                 scale=SCALE)
                        else:
                            nc.vector.tensor_scalar(
                                pt2.bitcast(mybir.dt.int16)[:], st2[:],
                                SCH_A, SCH_B,
                                mybir.AluOpType.mult, mybir.AluOpType.add)

                    def emit_av_pair(m, j):
                        pts = pts_by_m[m]
                        pt2 = pts.pop(j)
                        if not pts:
                            del pts_by_m[m]
                        last = m == 31
                        for b in range(2):  # two col-groups run concurrently
                            a = 2 * j + b
                            H = 4 * hg + a
                            vs = slice(33 * H, 33 * H + 33)
                            ps = slice(512 * b, 512 * (b + 1))
                            nc.tensor.matmul(
                                Op[j][64 * b:64 * b + 33, :],
                                lhsT=Vsb[m][:, vs], rhs=pt2[:, ps],
                                start=False, stop=last,
                                tile_position=(0, 64 * b), skip_group_check=True)

                    emit_s_pair(0, 0)
                    emit_s_pair(0, 1)
                    # previous phase's denominator/normalize tail lands here:
                    # behind this phase's first score MMs in the PE queue, so
                    # the PE streams scores while VectorE drains the chain
                    # (keeps PE idle below the ~3.4us HAM re-throttle window)
                    if prev_tail is not None:
                        prev_tail()
                    emit_zero()
                    for m in range(32):
                        if m + 1 < 32:
                            emit_s_pair(m + 1, 0)
                        emit_av_pair(m, 0)
                        if m + 1 < 32:
                            emit_s_pair(m + 1, 1)
                        emit_av_pair(m, 1)
                        if inject is not None and m == inject_m:
                            inject()

                    def tail():
                        # denominators sit at rows 32 (head A) and 96 (head B)
                        # of each pair bank; copy out, PE-broadcast into the
                        # spare rows 32-63 / 96-127, reciprocal, then
                        # normalize into Osb
                        for j in range(2):
                            d4s = []
                            for b in range(2):
                                d4 = db.tile([1, 512], F32, name=f"d4_{j}{b}",
                                             tag=f"d4_{j}{b}", bufs=2)
                                nc.vector.tensor_copy(
                                    d4[:], Op[j][64 * b + 32:64 * b + 33, :])
                                d4s.append(d4)
                            for b in range(2):
                                nc.tensor.matmul(
                                    Op[j][64 * b + 32:64 * b + 64, :],
                                    lhsT=ONES32F[:],
                                    rhs=d4s[b][:], start=True, stop=True,
                                    tile_position=(0, 64 * b + 32),
                                    skip_group_check=True)
                            rj = rb.tile([128, 512], F32, name=f"rj{j}",
                                         tag=f"rj{j}", bufs=2)
                            nc.vector.reciprocal_approx_fast(
                                out=rj[:], in_=Op[j][:, :])
                            for b in range(2):
                                a = 2 * j + b
                                nc.vector.tensor_tensor(
                                    Osb[hg][32 * a:32 * a + 32, qs],
                                    Op[j][64 * b:64 * b + 32, :],
                                    rj[64 * b + 32:64 * b + 64, :],
                                    mybir.AluOpType.mult)

                    return tail

            t = run_phase(0, 0)
            t = run_phase(1, 0, prev_tail=t)
            # qh=0 output projection injected into the next phase's m-loop
            # (by m=3 the qh=0 normalize chain has drained; the PE keeps
            # streaming scores in the meantime)
            t = run_phase(0, 1, prev_tail=t, inject=lambda: emit_oproj(0))
            t = run_phase(1, 1, prev_tail=t)
            t()
            emit_oproj(1)


_NC = None


def _get_program():
    global _NC
    if _NC is None:
        _NC = _build_program()
    return _NC


def kernel(x, qkv_w, proj_w, gamma, _trace=False):
    """Full inputs in, full output out. Shards across 8 NeuronCores internally."""
    nc = _get_program()
    B = x.shape[0]
    xf = np.ascontiguousarray(x.reshape(B, C, N).astype(np.float32))
    xf_bf = xf.astype(NPBF16)

    wqT = np.ascontiguousarray(qkv_w[0:256].T.astype(NPBF16))
    wkT = np.ascontiguousarray(qkv_w[256:512].T.astype(NPBF16))
    wvT = np.ascontiguousarray(qkv_w[512:768].T.astype(NPBF16))
    pjT = np.ascontiguousarray(proj_w.T.astype(NPBF16))
    gam = np.full((128, 1), np.float32(gamma.reshape(-1)[0]), dtype=np.float32)

    in_maps = []
    for core in range(8):
        b, qi = divmod(core, 4)
        qs = slice(Q * qi, Q * (qi + 1))
        in_maps.append({
            "xb": xf_bf[b],
            "xqb": np.ascontiguousarray(xf_bf[b][:, qs]),
            "xq": np.ascontiguousarray(xf[b][:, qs]),
            "wkT": wkT, "wqT": wqT, "wvT": wvT, "pjT": pjT,
            "gam": gam,
        })

    res = run_bass_kernel_spmd(nc, in_maps, core_ids=list(range(8)), trace=_trace)

    outf = np.empty((B, C, N), dtype=np.float32)
    for core in range(8):
        b, qi = divmod(core, 4)
        outf[b][:, Q * qi:Q * (qi + 1)] = res.results[core]["out"]
    result = outf.reshape(x.shape)
    if _trace:
        return result, res
    return result



# revision 17
# speedup vs baseline: 1.1312x; 1.1312x over previous
"""Multi-head self-attention 2d (B=2, C=256, H=W=64, 8 heads x 32 dim) on 8 TRN2 cores.

Sharding: batch (2-way) x query-rows-of-N=H*W (4-way) => 8 cores, no collectives.
Each core computes, for its (batch b, query shard of 1024 rows):
  - K', V projections over the FULL 4096 positions of its batch (4x replicated work,
    but removes all cross-core communication),
  - Q' projection for its 1024 query rows,
  - S^T = K'^T-layout scores via tensor-engine row-packing (contraction d=32, 4 heads
    concurrently in the 128x128 PE array),
  - softmax without max-subtraction (scores are ~N(0,1); exp on ScalarE, denominator
    via ones-matmul, division on VectorE),
  - attn @ V via col-packed matmuls accumulating over the 4096 positions,
  - output projection, then out = gamma * proj + x on its [256, 1024] slice.
"""

import os
import sys

import numpy as np

for _p in ("/opt/trn_rl_repo", "/root/.axon_site/_ro/trn_rl_repo"):
    if os.path.isdir(_p) and _p not in sys.path:
        sys.path.insert(0, _p)

import ml_dtypes
import concourse.bacc as bacc
import concourse.bass as bass
import concourse.tile as tile
from concourse import mybir
from concourse.bass_utils import run_bass_kernel_spmd

BF16 = mybir.dt.bfloat16
F32 = mybir.dt.float32
NPBF16 = ml_dtypes.bfloat16

NH, D = 8, 32          # heads, head dim
C = 256                # channels
N = 4096               # H*W positions
Q = 1024               # query shard per core
SCALE = 1.0 / np.sqrt(D)

# Schraudolph bf16 exp on the vector engine: bf16_bits(exp(y)) ~= round(y*128/ln2
# + (127*128 - c)). Fold the attention scale into the multiplier. c tuned for
# min max-rel-error under round-to-nearest (~3.3%).
SCH_A = float(SCALE * 128.0 / np.log(2.0))
SCH_B = float(127.0 * 128.0 - 5.5)
# Fraction of exp tiles computed on ScalarE (rest on VectorE via Schraudolph),
# chosen to balance the two engines' total busy time (measured: ACT 1115ns,
# TS 1222ns per [128,1024] tile, plus ~90ns/m of other VectorE work).
ACT_UNITS_PER_32 = 17


def _build_program():
    nc = bacc.Bacc("TRN2", target_bir_lowering=False, debug=False)

    xb = nc.dram_tensor("xb", [C, N], BF16, kind="ExternalInput")
    xqb = nc.dram_tensor("xqb", [C, Q], BF16, kind="ExternalInput")
    xq = nc.dram_tensor("xq", [C, Q], F32, kind="ExternalInput")
    wkT = nc.dram_tensor("wkT", [C, C], BF16, kind="ExternalInput")
    wqT = nc.dram_tensor("wqT", [C, C], BF16, kind="ExternalInput")
    wvT = nc.dram_tensor("wvT", [C, C], BF16, kind="ExternalInput")
    pjT = nc.dram_tensor("pjT", [C, C], BF16, kind="ExternalInput")
    gam = nc.dram_tensor("gam", [128, 1], F32, kind="ExternalInput")
    out = nc.dram_tensor("out", [C, Q], F32, kind="ExternalOutput")

    with tile.TileContext(nc) as tc:
        _emit(tc, xb, xqb, xq, wkT, wqT, wvT, pjT, gam, out)
    nc.compile()
    return nc


def _emit(tc, xb, xqb, xq, wkT, wqT, wvT, pjT, gam, out):
    from contextlib import ExitStack

    nc = tc.nc
    Exp = mybir.ActivationFunctionType.Exp

    with ExitStack() as ctx:
        per = ctx.enter_context(tc.tile_pool(name="persist", bufs=1))

        def ptile(name, shape, dtype):
            return per.tile(shape, dtype, name=name, tag=name)

        XB = [ptile(f"XB{i}", [128, N], BF16) for i in range(2)]
        XQB = [ptile(f"XQB{i}", [128, Q], BF16) for i in range(2)]
        XQ = [ptile(f"XQ{i}", [128, Q], F32) for i in range(2)]
        WK = [ptile(f"WK{i}", [128, C], BF16) for i in range(2)]
        WQ = [ptile(f"WQ{i}", [128, C], BF16) for i in range(2)]
        WV = [ptile(f"WV{i}", [128, C], BF16) for i in range(2)]
        PJ = [ptile(f"PJ{i}", [128, C], BF16) for i in range(2)]
        G = ptile("G", [128, 1], F32)
        ONE = ptile("ONE", [128, 1], BF16)
        Z128 = ptile("Z128", [1, 128], BF16)   # zeros, lhsT of bank-zeroing matmul
        ONES512 = ptile("ONES512", [1, 512], BF16)
        ONES32F = ptile("ONES32F", [1, 32], F32)  # lhsT of denominator-broadcast matmul
        Ksb = [ptile(f"Ksb{i}", [128, N], BF16) for i in range(2)]
        Qsb = [ptile(f"Qsb{i}", [128, Q], BF16) for i in range(2)]
        Vsb = [ptile(f"Vsb{m}", [128, NH * 33], BF16) for m in range(32)]
        Osb = [ptile(f"Osb{i}", [128, Q], BF16) for i in range(2)]

        # small critical tensors first so Q'/K' projections start ASAP
        for i in range(2):
            r = slice(128 * i, 128 * (i + 1))
            nc.sync.dma_start(WQ[i][:], wqT[r, :])
            nc.sync.dma_start(XQB[i][:], xqb[r, :])
            nc.sync.dma_start(WK[i][:], wkT[r, :])
            nc.sync.dma_start(WV[i][:], wvT[r, :])
            nc.sync.dma_start(PJ[i][:], pjT[r, :])
        nc.sync.dma_start(G[:], gam[:, :])
        for ch in range(4):  # chunked so projections can start early
            cs_ = slice(1024 * ch, 1024 * (ch + 1))
            for i in range(2):
                r = slice(128 * i, 128 * (i + 1))
                nc.sync.dma_start(XB[i][:, cs_], xb[r, cs_])
        for i in range(2):
            r = slice(128 * i, 128 * (i + 1))
            nc.sync.dma_start(XQ[i][:], xq[r, :])  # only needed at the end
        nc.gpsimd.memset(ONE[:], 1.0)
        nc.gpsimd.memset(Z128[:], 0.0)
        nc.gpsimd.memset(ONES512[:], 1.0)
        nc.gpsimd.memset(ONES32F[:], 1.0)

        # ---- projections: Q'[hd, q], K'[hd, m], V[m, hd] --------------------
        # PSUM->SBUF copies alternate ScalarE/VectorE so neither engine gates
        # the projection phase on its own.
        with tc.tile_pool(name="pp", bufs=4, space="PSUM") as pp:
            ncopy = 0

            def pcopy(dst, src):
                nonlocal ncopy
                if ncopy % 2 == 0:
                    nc.vector.tensor_copy(dst, src)
                else:
                    nc.scalar.copy(dst, src)
                ncopy += 1

            for p in range(2):
                hs = slice(128 * p, 128 * (p + 1))
                for t in range(2):
                    qp = pp.tile([128, 512], F32, name="qp", tag="pp")
                    ts_ = slice(512 * t, 512 * (t + 1))
                    nc.tensor.matmul(qp[:], lhsT=WQ[0][:, hs], rhs=XQB[0][:, ts_],
                                     start=True, stop=False)
                    nc.tensor.matmul(qp[:], lhsT=WQ[1][:, hs], rhs=XQB[1][:, ts_],
                                     start=False, stop=True)
                    pcopy(Qsb[p][:, ts_], qp[:])
            # K' and V interleaved chunk-major so the PE tracks the XB DMA
            # stream instead of waiting for the full tensor before V starts
            for ch in range(4):
                for t in (2 * ch, 2 * ch + 1):
                    ts_ = slice(512 * t, 512 * (t + 1))
                    for p in range(2):
                        hs = slice(128 * p, 128 * (p + 1))
                        kp = pp.tile([128, 512], F32, name="kp", tag="pp")
                        nc.tensor.matmul(kp[:], lhsT=WK[0][:, hs],
                                         rhs=XB[0][:, ts_],
                                         start=True, stop=False)
                        nc.tensor.matmul(kp[:], lhsT=WK[1][:, hs],
                                         rhs=XB[1][:, ts_],
                                         start=False, stop=True)
                        pcopy(Ksb[p][:, ts_], kp[:])
                for m in range(8 * ch, 8 * ch + 8):
                    vp = pp.tile([128, 512], F32, name="vp", tag="pp")
                    ms = slice(128 * m, 128 * (m + 1))
                    nc.tensor.matmul(vp[:, :C], lhsT=XB[0][:, ms], rhs=WV[0][:],
                                     start=True, stop=False)
                    nc.tensor.matmul(vp[:, :C], lhsT=XB[1][:, ms], rhs=WV[1][:],
                                     start=False, stop=True)
                    v3 = Vsb[m].rearrange("p (h w) -> p h w", w=33)
                    pcopy(v3[:, :, 0:32],
                          vp[:, :C].rearrange("p (h w) -> p h w", w=32))
                    nc.gpsimd.memset(v3[:, :, 32:33], 1.0)

        # ---- attention ------------------------------------------------------
        with ExitStack() as actx:
            sp = actx.enter_context(tc.tile_pool(name="sp", bufs=3, space="PSUM"))
            opl = actx.enter_context(tc.tile_pool(name="opl", bufs=1, space="PSUM"))
            pb = actx.enter_context(tc.tile_pool(name="pb", bufs=4))
            db = actx.enter_context(tc.tile_pool(name="db", bufs=8))
            rb = actx.enter_context(tc.tile_pool(name="rb", bufs=2))
            ob = actx.enter_context(tc.tile_pool(name="ob", bufs=4))

            def emit_oproj(qh):
                # output projection + residual for one q-half; borrows an sp
                # score slot for PSUM so it can overlap a running phase
                qs = slice(512 * qh, 512 * (qh + 1))
                pjp = sp.tile([128, 1024], F32, name="pjp", tag="st2")
                for ct in range(2):
                    cs = slice(128 * ct, 128 * (ct + 1))
                    pp2 = pjp[:, 512 * ct:512 * (ct + 1)]
                    nc.tensor.matmul(pp2, lhsT=PJ[0][:, cs], rhs=Osb[0][:, qs],
                                     start=True, stop=False,
                                     skip_group_check=True)
                    nc.tensor.matmul(pp2, lhsT=PJ[1][:, cs], rhs=Osb[1][:, qs],
                                     start=False, stop=True,
                                     skip_group_check=True)
                    obt = ob.tile([128, 512], F32, name="obt", tag="obt")
                    nc.vector.scalar_tensor_tensor(
                        obt[:], pp2, G[:], XQ[ct][:, qs],
                        mybir.AluOpType.mult, mybir.AluOpType.add)
                    nc.sync.dma_start(out[cs, qs], obt[:])

            # qh outer so the qh=0 output projection can overlap the qh=1
            # phases; hg inner.
            def run_phase(hg, qh, prev_tail=None, inject=None, inject_m=3):
                    qs = slice(512 * qh, 512 * (qh + 1))
                    Op = [opl.tile([128, 512], F32, name=f"Op{j}", tag=f"Op{j}")
                          for j in range(2)]

                    def emit_zero():
                        # hardware start=True only zeroes the region an MM
                        # writes; partial-width accumulation chains must land
                        # on explicitly zeroed banks (stale PSUM junk leaks
                        # in otherwise)
                        for j in range(2):
                            nc.tensor.matmul(Op[j][:], lhsT=Z128[:],
                                             rhs=ONES512[:], start=True,
                                             stop=True, skip_group_check=True)
                    emit_zero()
                    # software-pipelined: emit the score quad for m+1 before
                    # the AV quad for m, so the PE never waits on exp results
                    # (sustained dense PE bursts also keep the HAM at 2.4GHz)
                    pts_by_m = {}

                    def emit_s(m):
                        ms = slice(128 * m, 128 * (m + 1))
                        sts = []
                        for g in range(2):
                            st2 = sp.tile([128, 1024], F32, name="st2",
                                          tag="st2")
                            sts.append(st2)
                        for g in range(2):
                            for j in range(2):
                                a = 2 * g + j
                                hh = slice(32 * a, 32 * (a + 1))
                                nc.tensor.matmul(
                                    sts[g][:, 512 * j:512 * (j + 1)],
                                    lhsT=Ksb[hg][hh, ms], rhs=Qsb[hg][hh, qs],
                                    start=True, stop=True,
                                    tile_position=(32 * a, 0))
                        pts = []
                        for g in range(2):
                            pt2 = pb.tile([128, 1024], BF16, name="pt2",
                                          tag="pt2")
                            pts.append(pt2)
                            # last 3 m-tiles -> ScalarE (frees VectorE for the
                            # denominator/normalize chain at the boundary);
                            # first 2 m-tiles -> VectorE (ScalarE drains its
                            # boundary backlog)
                            k = 2 * m + g
                            if m >= 29:
                                on_scalar = True
                            elif m < 2:
                                on_scalar = False
                            else:
                                on_scalar = (k * ACT_UNITS_PER_32) % 32 \
                                    < ACT_UNITS_PER_32
                            if on_scalar:
                                nc.scalar.activation(pt2[:], sts[g][:], Exp,
                                                     scale=SCALE)
                            else:
                                nc.vector.tensor_scalar(
                                    pt2.bitcast(mybir.dt.int16)[:], sts[g][:],
                                    SCH_A, SCH_B,
                                    mybir.AluOpType.mult, mybir.AluOpType.add)
                        pts_by_m[m] = pts

                    def emit_av(m):
                        pts = pts_by_m.pop(m)
                        last = m == 31
                        # pair the two col-groups per j so they run
                        # concurrently, and finish pts[0] consumers first
                        for j, b in ((0, 0), (0, 1), (1, 0), (1, 1)):
                            a = 2 * j + b
                            H = 4 * hg + a
                            vs = slice(33 * H, 33 * H + 33)
                            ps = slice(512 * b, 512 * (b + 1))
                            nc.tensor.matmul(
                                Op[j][64 * b:64 * b + 33, :],
                                lhsT=Vsb[m][:, vs], rhs=pts[j][:, ps],
                                start=False, stop=last,
                                tile_position=(0, 64 * b),
                                skip_group_check=True)

                    emit_s(0)
                    for m in range(32):
                        if m + 1 < 32:
                            emit_s(m + 1)
                        emit_av(m)
                        if inject is not None and m == inject_m:
                            inject()
                    # denominators sit at rows 32 (head A) and 96 (head B) of
                    # each pair bank; copy out, PE-broadcast into the spare
                    # rows 32-63 / 96-127, reciprocal, then normalize into Osb
                    for j in range(2):
                        d4s = []
                        for b in range(2):
                            d4 = db.tile([1, 512], F32, name=f"d4_{j}{b}",
                                         tag=f"d4_{j}{b}", bufs=2)
                            nc.vector.tensor_copy(
                                d4[:], Op[j][64 * b + 32:64 * b + 33, :])
                            d4s.append(d4)
                        for b in range(2):
                            nc.tensor.matmul(
                                Op[j][64 * b + 32:64 * b + 64, :],
                                lhsT=ONES32F[:],
                                rhs=d4s[b][:], start=True, stop=True,
                                tile_position=(0, 64 * b + 32),
                                skip_group_check=True)
                        rj = rb.tile([128, 512], F32, name=f"rj{j}",
                                     tag=f"rj{j}", bufs=2)
                        nc.vector.reciprocal_approx_fast(
                            out=rj[:], in_=Op[j][:, :])
                        for b in range(2):
                            a = 2 * j + b
                            nc.vector.tensor_tensor(
                                Osb[hg][32 * a:32 * a + 32, qs],
                                Op[j][64 * b:64 * b + 32, :],
                                rj[64 * b + 32:64 * b + 64, :],
                                mybir.AluOpType.mult)

            run_phase(0, 0)
            run_phase(1, 0)
            # qh=0 output projection injected into the next phase's m-loop
            # (by m=3 the qh=0 normalize chain has drained; the PE keeps
            # streaming scores in the meantime)
            run_phase(0, 1, inject=lambda: emit_oproj(0))
            run_phase(1, 1)
            emit_oproj(1)


_NC = None


def _get_program():
    global _NC
    if _NC is None:
        _NC = _build_program()
    return _NC


def kernel(x, qkv_w, proj_w, gamma, _trace=False):
    """Full inputs in, full output out. Shards across 8 NeuronCores internally."""
    nc = _get_program()
    B = x.shape[0]
    xf = np.ascontiguousarray(x.reshape(B, C, N).astype(np.float32))
    xf_bf = xf.astype(NPBF16)

    wqT = np.ascontiguousarray(qkv_w[0:256].T.astype(NPBF16))
    wkT = np.ascontiguousarray(qkv_w[256:512].T.astype(NPBF16))
    wvT = np.ascontiguousarray(qkv_w[512:768].T.astype(NPBF16))
    pjT = np.ascontiguousarray(proj_w.T.astype(NPBF16))
    gam = np.full((128, 1), np.float32(gamma.reshape(-1)[0]), dtype=np.float32)

    in_maps = []
    for core in range(8):
        b, qi = divmod(core, 4)
        qs = slice(Q * qi, Q * (qi + 1))
        in_maps.append({
            "xb": xf_bf[b],
            "xqb": np.ascontiguousarray(xf_bf[b][:, qs]),
            "xq": np.ascontiguousarray(xf[b][:, qs]),
            "wkT": wkT, "wqT": wqT, "wvT": wvT, "pjT": pjT,
            "gam": gam,
        })

    res = run_bass_kernel_spmd(nc, in_maps, core_ids=list(range(8)), trace=_trace)

    outf = np.empty((B, C, N), dtype=np.float32)
    for core in range(8):
        b, qi = divmod(core, 4)
        outf[b][:, Q * qi:Q * (qi + 1)] = res.results[core]["out"]
    result = outf.reshape(x.shape)
    if _trace:
        return result, res
    return result



# revision 18
# speedup vs baseline: 1.3759x; 1.2163x over previous
"""Multi-head self-attention 2d (B=2, C=256, H=W=64, 8 heads x 32 dim) on 8 TRN2 cores.

Sharding: batch (2-way) x query-rows-of-N=H*W (4-way) => 8 cores, no collectives.
Each core computes, for its (batch b, query shard of 1024 rows):
  - K', V projections over the FULL 4096 positions of its batch (4x replicated work,
    but removes all cross-core communication),
  - Q' projection for its 1024 query rows,
  - S^T = K'^T-layout scores via tensor-engine row-packing (contraction d=32, 4 heads
    concurrently in the 128x128 PE array),
  - softmax without max-subtraction (scores are ~N(0,1); exp on ScalarE, denominator
    via ones-matmul, division on VectorE),
  - attn @ V via col-packed matmuls accumulating over the 4096 positions,
  - output projection, then out = gamma * proj + x on its [256, 1024] slice.
"""

import os
import sys

import numpy as np

for _p in ("/opt/trn_rl_repo", "/root/.axon_site/_ro/trn_rl_repo"):
    if os.path.isdir(_p) and _p not in sys.path:
        sys.path.insert(0, _p)

import ml_dtypes
import concourse.bacc as bacc
import concourse.bass as bass
import concourse.tile as tile
from concourse import mybir
from concourse.bass_utils import run_bass_kernel_spmd

BF16 = mybir.dt.bfloat16
F32 = mybir.dt.float32
NPBF16 = ml_dtypes.bfloat16

NH, D = 8, 32          # heads, head dim
C = 256                # channels
N = 4096               # H*W positions
Q = 1024               # query shard per core
SCALE = 1.0 / np.sqrt(D)

# Schraudolph bf16 exp on the vector engine: bf16_bits(exp(y)) ~= round(y*128/ln2
# + (127*128 - c)). Fold the attention scale into the multiplier. c tuned for
# min max-rel-error under round-to-nearest (~3.3%).
SCH_A = float(SCALE * 128.0 / np.log(2.0))
SCH_B = float(127.0 * 128.0 - 5.5)
# Fraction of exp tiles computed on ScalarE (rest on VectorE via Schraudolph),
# chosen to balance the two engines' total busy time (measured: ACT 1115ns,
# TS 1222ns per [128,1024] tile, plus ~90ns/m of other VectorE work).
ACT_UNITS_PER_32 = 20


def _build_program():
    nc = bacc.Bacc("TRN2", target_bir_lowering=False, debug=False)

    xb = nc.dram_tensor("xb", [C, N], BF16, kind="ExternalInput")
    xqb = nc.dram_tensor("xqb", [C, Q], BF16, kind="ExternalInput")
    xq = nc.dram_tensor("xq", [C, Q], F32, kind="ExternalInput")
    wkT = nc.dram_tensor("wkT", [C, C], BF16, kind="ExternalInput")
    wqT = nc.dram_tensor("wqT", [C, C], BF16, kind="ExternalInput")
    wvT = nc.dram_tensor("wvT", [C, C], BF16, kind="ExternalInput")
    pjT = nc.dram_tensor("pjT", [C, C], BF16, kind="ExternalInput")
    gam = nc.dram_tensor("gam", [128, 1], F32, kind="ExternalInput")
    out = nc.dram_tensor("out", [C, Q], F32, kind="ExternalOutput")

    with tile.TileContext(nc) as tc:
        _emit(tc, xb, xqb, xq, wkT, wqT, wvT, pjT, gam, out)
    nc.compile()
    return nc


def _emit(tc, xb, xqb, xq, wkT, wqT, wvT, pjT, gam, out):
    from contextlib import ExitStack

    nc = tc.nc
    Exp = mybir.ActivationFunctionType.Exp

    with ExitStack() as ctx:
        per = ctx.enter_context(tc.tile_pool(name="persist", bufs=1))

        def ptile(name, shape, dtype):
            return per.tile(shape, dtype, name=name, tag=name)

        XB = [ptile(f"XB{i}", [128, N], BF16) for i in range(2)]
        XQB = [ptile(f"XQB{i}", [128, Q], BF16) for i in range(2)]
        XQ = [ptile(f"XQ{i}", [128, Q], F32) for i in range(2)]
        WK = [ptile(f"WK{i}", [128, C], BF16) for i in range(2)]
        WQ = [ptile(f"WQ{i}", [128, C], BF16) for i in range(2)]
        WV = [ptile(f"WV{i}", [128, C], BF16) for i in range(2)]
        PJ = [ptile(f"PJ{i}", [128, C], BF16) for i in range(2)]
        G = ptile("G", [128, 1], F32)
        ONE = ptile("ONE", [128, 1], BF16)
        Z128 = ptile("Z128", [1, 128], BF16)   # zeros, lhsT of bank-zeroing matmul
        ONES512 = ptile("ONES512", [1, 512], BF16)
        ONES32F = ptile("ONES32F", [1, 32], F32)  # lhsT of denominator-broadcast matmul
        Ksb = [ptile(f"Ksb{i}", [128, N], BF16) for i in range(2)]
        Qsb = [ptile(f"Qsb{i}", [128, Q], BF16) for i in range(2)]
        Vsb = [ptile(f"Vsb{m}", [128, NH * 33], BF16) for m in range(32)]
        Osb = [ptile(f"Osb{i}", [128, Q], BF16) for i in range(2)]

        # small critical tensors first so Q'/K' projections start ASAP
        for i in range(2):
            r = slice(128 * i, 128 * (i + 1))
            nc.sync.dma_start(WQ[i][:], wqT[r, :])
            nc.sync.dma_start(XQB[i][:], xqb[r, :])
            nc.sync.dma_start(WK[i][:], wkT[r, :])
            nc.sync.dma_start(WV[i][:], wvT[r, :])
            nc.sync.dma_start(PJ[i][:], pjT[r, :])
        nc.sync.dma_start(G[:], gam[:, :])
        for ch in range(4):  # chunked so projections can start early
            cs_ = slice(1024 * ch, 1024 * (ch + 1))
            for i in range(2):
                r = slice(128 * i, 128 * (i + 1))
                nc.sync.dma_start(XB[i][:, cs_], xb[r, cs_])
        for i in range(2):
            r = slice(128 * i, 128 * (i + 1))
            nc.sync.dma_start(XQ[i][:], xq[r, :])  # only needed at the end
        nc.vector.memset(ONE[:], 1.0)
        nc.vector.memset(Z128[:], 0.0)
        nc.vector.memset(ONES512[:], 1.0)
        nc.vector.memset(ONES32F[:], 1.0)

        # ---- projections: Q'[hd, q], K'[hd, m], V[m, hd] --------------------
        # PSUM->SBUF copies alternate ScalarE/VectorE so neither engine gates
        # the projection phase on its own.
        with tc.tile_pool(name="pp", bufs=4, space="PSUM") as pp:
            ncopy = 0

            def pcopy(dst, src):
                nonlocal ncopy
                if ncopy % 2 == 0:
                    nc.vector.tensor_copy(dst, src)
                else:
                    nc.scalar.copy(dst, src)
                ncopy += 1

            for p in range(2):
                hs = slice(128 * p, 128 * (p + 1))
                for t in range(2):
                    qp = pp.tile([128, 512], F32, name="qp", tag="pp")
                    ts_ = slice(512 * t, 512 * (t + 1))
                    nc.tensor.matmul(qp[:], lhsT=WQ[0][:, hs], rhs=XQB[0][:, ts_],
                                     start=True, stop=False)
                    nc.tensor.matmul(qp[:], lhsT=WQ[1][:, hs], rhs=XQB[1][:, ts_],
                                     start=False, stop=True)
                    pcopy(Qsb[p][:, ts_], qp[:])
            for t in range(8):
                ts_ = slice(512 * t, 512 * (t + 1))
                for p in range(2):
                    hs = slice(128 * p, 128 * (p + 1))
                    kp = pp.tile([128, 512], F32, name="kp", tag="pp")
                    nc.tensor.matmul(kp[:], lhsT=WK[0][:, hs], rhs=XB[0][:, ts_],
                                     start=True, stop=False)
                    nc.tensor.matmul(kp[:], lhsT=WK[1][:, hs], rhs=XB[1][:, ts_],
                                     start=False, stop=True)
                    pcopy(Ksb[p][:, ts_], kp[:])
            for m in range(32):
                vp = pp.tile([128, 512], F32, name="vp", tag="pp")
                ms = slice(128 * m, 128 * (m + 1))
                nc.tensor.matmul(vp[:, :C], lhsT=XB[0][:, ms], rhs=WV[0][:],
                                 start=True, stop=False)
                nc.tensor.matmul(vp[:, :C], lhsT=XB[1][:, ms], rhs=WV[1][:],
                                 start=False, stop=True)
                v3 = Vsb[m].rearrange("p (h w) -> p h w", w=33)
                pcopy(v3[:, :, 0:32],
                      vp[:, :C].rearrange("p (h w) -> p h w", w=32))
                nc.vector.memset(v3[:, :, 32:33], 1.0)

        # ---- attention ------------------------------------------------------
        with ExitStack() as actx:
            sp = actx.enter_context(tc.tile_pool(name="sp", bufs=3, space="PSUM"))
            opl = actx.enter_context(tc.tile_pool(name="opl", bufs=1, space="PSUM"))
            pb = actx.enter_context(tc.tile_pool(name="pb", bufs=4))
            db = actx.enter_context(tc.tile_pool(name="db", bufs=8))
            rb = actx.enter_context(tc.tile_pool(name="rb", bufs=2))
            ob = actx.enter_context(tc.tile_pool(name="ob", bufs=4))

            def emit_oproj(qh):
                # output projection + residual for one q-half; borrows an sp
                # score slot for PSUM so it can overlap a running phase
                qs = slice(512 * qh, 512 * (qh + 1))
                pjp = sp.tile([128, 1024], F32, name="pjp", tag="st2")
                for ct in range(2):
                    cs = slice(128 * ct, 128 * (ct + 1))
                    pp2 = pjp[:, 512 * ct:512 * (ct + 1)]
                    nc.tensor.matmul(pp2, lhsT=PJ[0][:, cs], rhs=Osb[0][:, qs],
                                     start=True, stop=False,
                                     skip_group_check=True)
                    nc.tensor.matmul(pp2, lhsT=PJ[1][:, cs], rhs=Osb[1][:, qs],
                                     start=False, stop=True,
                                     skip_group_check=True)
                    obt = ob.tile([128, 512], F32, name="obt", tag="obt")
                    nc.vector.scalar_tensor_tensor(
                        obt[:], pp2, G[:], XQ[ct][:, qs],
                        mybir.AluOpType.mult, mybir.AluOpType.add)
                    nc.sync.dma_start(out[cs, qs], obt[:])

            # qh outer so the qh=0 output projection can overlap the qh=1
            # phases; hg inner.
            def run_phase(hg, qh, prev_tail=None, inject=None, inject_m=3):
                    qs = slice(512 * qh, 512 * (qh + 1))
                    Op = [opl.tile([128, 512], F32, name=f"Op{j}", tag=f"Op{j}")
                          for j in range(2)]

                    def emit_zero():
                        # hardware start=True only zeroes the region an MM
                        # writes; partial-width accumulation chains must land
                        # on explicitly zeroed banks (stale PSUM junk leaks
                        # in otherwise)
                        for j in range(2):
                            nc.tensor.matmul(Op[j][:], lhsT=Z128[:],
                                             rhs=ONES512[:], start=True,
                                             stop=True, skip_group_check=True)
                    emit_zero()
                    # software-pipelined: emit the score quad for m+1 before
                    # the AV quad for m, so the PE never waits on exp results
                    # (sustained dense PE bursts also keep the HAM at 2.4GHz)
                    pts_by_m = {}

                    def emit_s(m):
                        ms = slice(128 * m, 128 * (m + 1))
                        sts = []
                        for g in range(2):
                            st2 = sp.tile([128, 1024], F32, name="st2",
                                          tag="st2")
                            sts.append(st2)
                        for g in range(2):
                            for j in range(2):
                                a = 2 * g + j
                                hh = slice(32 * a, 32 * (a + 1))
                                nc.tensor.matmul(
                                    sts[g][:, 512 * j:512 * (j + 1)],
                                    lhsT=Ksb[hg][hh, ms], rhs=Qsb[hg][hh, qs],
                                    start=True, stop=True,
                                    tile_position=(32 * a, 0))
                        pts = []
                        for g in range(2):
                            pt2 = pb.tile([128, 1024], BF16, name="pt2",
                                          tag="pt2")
                            pts.append(pt2)
                            k = 2 * m + g
                            on_scalar = (k * ACT_UNITS_PER_32) % 32 \
                                < ACT_UNITS_PER_32
                            if on_scalar:
                                nc.scalar.activation(pt2[:], sts[g][:], Exp,
                                                     scale=SCALE)
                            else:
                                nc.vector.tensor_scalar(
                                    pt2.bitcast(mybir.dt.int16)[:], sts[g][:],
                                    SCH_A, SCH_B,
                                    mybir.AluOpType.mult, mybir.AluOpType.add)
                        pts_by_m[m] = pts

                    def emit_av(m):
                        pts = pts_by_m.pop(m)
                        last = m == 31
                        # pair the two col-groups per j so they run
                        # concurrently, and finish pts[0] consumers first
                        for j, b in ((0, 0), (0, 1), (1, 0), (1, 1)):
                            a = 2 * j + b
                            H = 4 * hg + a
                            vs = slice(33 * H, 33 * H + 33)
                            ps = slice(512 * b, 512 * (b + 1))
                            nc.tensor.matmul(
                                Op[j][64 * b:64 * b + 33, :],
                                lhsT=Vsb[m][:, vs], rhs=pts[j][:, ps],
                                start=False, stop=last,
                                tile_position=(0, 64 * b),
                                skip_group_check=True)

                    emit_s(0)
                    for m in range(32):
                        if m + 1 < 32:
                            emit_s(m + 1)
                        emit_av(m)
                        if inject is not None and m == inject_m:
                            inject()
                    # denominators sit at rows 32 (head A) and 96 (head B) of
                    # each pair bank; copy out, PE-broadcast into the spare
                    # rows 32-63 / 96-127, reciprocal, then normalize into Osb
                    for j in range(2):
                        d4s = []
                        for b in range(2):
                            d4 = db.tile([1, 512], F32, name=f"d4_{j}{b}",
                                         tag=f"d4_{j}{b}", bufs=2)
                            nc.vector.tensor_copy(
                                d4[:], Op[j][64 * b + 32:64 * b + 33, :])
                            d4s.append(d4)
                        for b in range(2):
                            nc.tensor.matmul(
                                Op[j][64 * b + 32:64 * b + 64, :],
                                lhsT=ONES32F[:],
                                rhs=d4s[b][:], start=True, stop=True,
                                tile_position=(0, 64 * b + 32),
                                skip_group_check=True)
                        rj = rb.tile([128, 512], F32, name=f"rj{j}",
                                     tag=f"rj{j}", bufs=2)
                        nc.vector.reciprocal_approx_fast(
                            out=rj[:], in_=Op[j][:, :])
                        for b in range(2):
                            a = 2 * j + b
                            nc.vector.tensor_tensor(
                                Osb[hg][32 * a:32 * a + 32, qs],
                                Op[j][64 * b:64 * b + 32, :],
                                rj[64 * b + 32:64 * b + 64, :],
                                mybir.AluOpType.mult)

            run_phase(0, 0)
            run_phase(1, 0)
            # qh=0 output projection injected into the next phase's m-loop
            # (by m=3 the qh=0 normalize chain has drained; the PE keeps
            # streaming scores in the meantime)
            run_phase(0, 1, inject=lambda: emit_oproj(0))
            run_phase(1, 1)
            emit_oproj(1)


_NC = None


def _get_program():
    global _NC
    if _NC is None:
        _NC = _build_program()
    return _NC


def kernel(x, qkv_w, proj_w, gamma, _trace=False):
    """Full inputs in, full output out. Shards across 8 NeuronCores internally."""
    nc = _get_program()
    B = x.shape[0]
    xf = np.ascontiguousarray(x.reshape(B, C, N).astype(np.float32))
    xf_bf = xf.astype(NPBF16)

    wqT = np.ascontiguousarray(qkv_w[0:256].T.astype(NPBF16))
    wkT = np.ascontiguousarray(qkv_w[256:512].T.astype(NPBF16))
    wvT = np.ascontiguousarray(qkv_w[512:768].T.astype(NPBF16))
    pjT = np.ascontiguousarray(proj_w.T.astype(NPBF16))
    gam = np.full((128, 1), np.float32(gamma.reshape(-1)[0]), dtype=np.float32)

    in_maps = []
    for core in range(8):
        b, qi = divmod(core, 4)
        qs = slice(Q * qi, Q * (qi + 1))
        in_maps.append({
            "xb": xf_bf[b],
            "xqb": np.ascontiguousarray(xf_bf[b][:, qs]),
            "xq": np.ascontiguousarray(xf[b][:, qs]),
            "wkT": wkT, "wqT": wqT, "wvT": wvT, "pjT": pjT,
            "gam": gam,
        })

    res = run_bass_kernel_spmd(nc, in_maps, core_ids=list(range(8)), trace=_trace)

    outf = np.empty((B, C, N), dtype=np.float32)
    for core in range(8):
        b, qi = divmod(core, 4)
        outf[b][:, Q * qi:Q * (qi + 1)] = res.results[core]["out"]
    result = outf.reshape(x.shape)
    if _trace:
        return result, res
    return result



# revision 20
# speedup vs baseline: 1.3839x; 1.0058x over previous
"""Multi-head self-attention 2d (B=2, C=256, H=W=64, 8 heads x 32 dim) on 8 TRN2 cores.

Sharding: batch (2-way) x query-rows-of-N=H*W (4-way) => 8 cores, no collectives.
Each core computes, for its (batch b, query shard of 1024 rows):
  - K', V projections over the FULL 4096 positions of its batch (4x replicated work,
    but removes all cross-core communication),
  - Q' projection for its 1024 query rows,
  - S^T = K'^T-layout scores via tensor-engine row-packing (contraction d=32, 4 heads
    concurrently in the 128x128 PE array),
  - softmax without max-subtraction (scores are ~N(0,1); exp on ScalarE, denominator
    via ones-matmul, division on VectorE),
  - attn @ V via col-packed matmuls accumulating over the 4096 positions,
  - output projection, then out = gamma * proj + x on its [256, 1024] slice.
"""

import os
import sys

import numpy as np

for _p in ("/opt/trn_rl_repo", "/root/.axon_site/_ro/trn_rl_repo"):
    if os.path.isdir(_p) and _p not in sys.path:
        sys.path.insert(0, _p)

import ml_dtypes
import concourse.bacc as bacc
import concourse.bass as bass
import concourse.tile as tile
from concourse import mybir
from concourse.bass_utils import run_bass_kernel_spmd

BF16 = mybir.dt.bfloat16
F32 = mybir.dt.float32
NPBF16 = ml_dtypes.bfloat16

NH, D = 8, 32          # heads, head dim
C = 256                # channels
N = 4096               # H*W positions
Q = 1024               # query shard per core
SCALE = 1.0 / np.sqrt(D)

# Schraudolph bf16 exp on the vector engine: bf16_bits(exp(y)) ~= round(y*128/ln2
# + (127*128 - c)). Fold the attention scale into the multiplier. c tuned for
# min max-rel-error under round-to-nearest (~3.3%).
SCH_A = float(SCALE * 128.0 / np.log(2.0))
SCH_B = float(127.0 * 128.0 - 5.5)
# Fraction of exp tiles computed on ScalarE (rest on VectorE via Schraudolph),
# chosen to balance the two engines' total busy time (measured: ACT 1115ns,
# TS 1222ns per [128,1024] tile, plus ~90ns/m of other VectorE work).
ACT_UNITS_PER_32 = 18


def _build_program():
    nc = bacc.Bacc("TRN2", target_bir_lowering=False, debug=False)

    xb = nc.dram_tensor("xb", [C, N], BF16, kind="ExternalInput")
    xqb = nc.dram_tensor("xqb", [C, Q], BF16, kind="ExternalInput")
    xq = nc.dram_tensor("xq", [C, Q], F32, kind="ExternalInput")
    wkT = nc.dram_tensor("wkT", [C, C], BF16, kind="ExternalInput")
    wqT = nc.dram_tensor("wqT", [C, C], BF16, kind="ExternalInput")
    wvT = nc.dram_tensor("wvT", [C, C], BF16, kind="ExternalInput")
    pjT = nc.dram_tensor("pjT", [C, C], BF16, kind="ExternalInput")
    gam = nc.dram_tensor("gam", [128, 1], F32, kind="ExternalInput")
    out = nc.dram_tensor("out", [C, Q], F32, kind="ExternalOutput")

    with tile.TileContext(nc) as tc:
        _emit(tc, xb, xqb, xq, wkT, wqT, wvT, pjT, gam, out)
    nc.compile()
    return nc


def _emit(tc, xb, xqb, xq, wkT, wqT, wvT, pjT, gam, out):
    from contextlib import ExitStack

    nc = tc.nc
    Exp = mybir.ActivationFunctionType.Exp

    with ExitStack() as ctx:
        per = ctx.enter_context(tc.tile_pool(name="persist", bufs=1))

        def ptile(name, shape, dtype):
            return per.tile(shape, dtype, name=name, tag=name)

        XB = [ptile(f"XB{i}", [128, N], BF16) for i in range(2)]
        XQB = [ptile(f"XQB{i}", [128, Q], BF16) for i in range(2)]
        XQ = [ptile(f"XQ{i}", [128, Q], F32) for i in range(2)]
        WK = [ptile(f"WK{i}", [128, C], BF16) for i in range(2)]
        WQ = [ptile(f"WQ{i}", [128, C], BF16) for i in range(2)]
        WV = [ptile(f"WV{i}", [128, C], BF16) for i in range(2)]
        PJ = [ptile(f"PJ{i}", [128, C], BF16) for i in range(2)]
        G = ptile("G", [128, 1], F32)
        ONE = ptile("ONE", [128, 1], BF16)
        Z128 = ptile("Z128", [1, 128], BF16)   # zeros, lhsT of bank-zeroing matmul
        ONES512 = ptile("ONES512", [1, 512], BF16)
        ONES32F = ptile("ONES32F", [1, 32], F32)  # lhsT of denominator-broadcast matmul
        Ksb = [ptile(f"Ksb{i}", [128, N], BF16) for i in range(2)]
        Qsb = [ptile(f"Qsb{i}", [128, Q], BF16) for i in range(2)]
        Vsb = [ptile(f"Vsb{m}", [128, NH * 33], BF16) for m in range(32)]
        Osb = [ptile(f"Osb{i}", [128, Q], BF16) for i in range(2)]

        # small critical tensors first so Q'/K' projections start ASAP
        for i in range(2):
            r = slice(128 * i, 128 * (i + 1))
            nc.sync.dma_start(WQ[i][:], wqT[r, :])
            nc.sync.dma_start(XQB[i][:], xqb[r, :])
            nc.sync.dma_start(WK[i][:], wkT[r, :])
            nc.sync.dma_start(WV[i][:], wvT[r, :])
            nc.sync.dma_start(PJ[i][:], pjT[r, :])
        nc.sync.dma_start(G[:], gam[:, :])
        for ch in range(4):  # chunked so projections can start early
            cs_ = slice(1024 * ch, 1024 * (ch + 1))
            for i in range(2):
                r = slice(128 * i, 128 * (i + 1))
                nc.sync.dma_start(XB[i][:, cs_], xb[r, cs_])
        for i in range(2):
            r = slice(128 * i, 128 * (i + 1))
            nc.sync.dma_start(XQ[i][:], xq[r, :])  # only needed at the end
        nc.vector.memset(ONE[:], 1.0)
        nc.vector.memset(Z128[:], 0.0)
        nc.vector.memset(ONES512[:], 1.0)
        nc.vector.memset(ONES32F[:], 1.0)

        # ---- projections: Q'[hd, q], K'[hd, m], V[m, hd] --------------------
        # PSUM->SBUF copies alternate ScalarE/VectorE so neither engine gates
        # the projection phase on its own.
        with tc.tile_pool(name="pp", bufs=4, space="PSUM") as pp:
            ncopy = 0

            def pcopy(dst, src):
                nonlocal ncopy
                if ncopy % 2 == 0:
                    nc.vector.tensor_copy(dst, src)
                else:
                    nc.scalar.copy(dst, src)
                ncopy += 1

            for p in range(2):
                hs = slice(128 * p, 128 * (p + 1))
                for t in range(2):
                    qp = pp.tile([128, 512], F32, name="qp", tag="pp")
                    ts_ = slice(512 * t, 512 * (t + 1))
                    nc.tensor.matmul(qp[:], lhsT=WQ[0][:, hs], rhs=XQB[0][:, ts_],
                                     start=True, stop=False)
                    nc.tensor.matmul(qp[:], lhsT=WQ[1][:, hs], rhs=XQB[1][:, ts_],
                                     start=False, stop=True)
                    pcopy(Qsb[p][:, ts_], qp[:])
            for t in range(8):
                ts_ = slice(512 * t, 512 * (t + 1))
                for p in range(2):
                    hs = slice(128 * p, 128 * (p + 1))
                    kp = pp.tile([128, 512], F32, name="kp", tag="pp")
                    nc.tensor.matmul(kp[:], lhsT=WK[0][:, hs], rhs=XB[0][:, ts_],
                                     start=True, stop=False)
                    nc.tensor.matmul(kp[:], lhsT=WK[1][:, hs], rhs=XB[1][:, ts_],
                                     start=False, stop=True)
                    pcopy(Ksb[p][:, ts_], kp[:])
            for m in range(32):
                vp = pp.tile([128, 512], F32, name="vp", tag="pp")
                ms = slice(128 * m, 128 * (m + 1))
                nc.tensor.matmul(vp[:, :C], lhsT=XB[0][:, ms], rhs=WV[0][:],
                                 start=True, stop=False)
                nc.tensor.matmul(vp[:, :C], lhsT=XB[1][:, ms], rhs=WV[1][:],
                                 start=False, stop=True)
                v3 = Vsb[m].rearrange("p (h w) -> p h w", w=33)
                pcopy(v3[:, :, 0:32],
                      vp[:, :C].rearrange("p (h w) -> p h w", w=32))
                nc.vector.memset(v3[:, :, 32:33], 1.0)

        # ---- attention ------------------------------------------------------
        with ExitStack() as actx:
            sp = actx.enter_context(tc.tile_pool(name="sp", bufs=3, space="PSUM"))
            opl = actx.enter_context(tc.tile_pool(name="opl", bufs=1, space="PSUM"))
            pb = actx.enter_context(tc.tile_pool(name="pb", bufs=4))
            db = actx.enter_context(tc.tile_pool(name="db", bufs=8))
            rb = actx.enter_context(tc.tile_pool(name="rb", bufs=2))
            ob = actx.enter_context(tc.tile_pool(name="ob", bufs=4))

            def emit_oproj(qh):
                # output projection + residual for one q-half; borrows an sp
                # score slot for PSUM so it can overlap a running phase
                qs = slice(512 * qh, 512 * (qh + 1))
                pjp = sp.tile([128, 1024], F32, name="pjp", tag="st2")
                for ct in range(2):
                    cs = slice(128 * ct, 128 * (ct + 1))
                    pp2 = pjp[:, 512 * ct:512 * (ct + 1)]
                    nc.tensor.matmul(pp2, lhsT=PJ[0][:, cs], rhs=Osb[0][:, qs],
                                     start=True, stop=False,
                                     skip_group_check=True)
                    nc.tensor.matmul(pp2, lhsT=PJ[1][:, cs], rhs=Osb[1][:, qs],
                                     start=False, stop=True,
                                     skip_group_check=True)
                    obt = ob.tile([128, 512], F32, name="obt", tag="obt")
                    nc.vector.scalar_tensor_tensor(
                        obt[:], pp2, G[:], XQ[ct][:, qs],
                        mybir.AluOpType.mult, mybir.AluOpType.add)
                    nc.sync.dma_start(out[cs, qs], obt[:])

            # qh outer so the qh=0 output projection can overlap the qh=1
            # phases; hg inner.
            def run_phase(hg, qh, prev_tail=None, inject=None, inject_m=3):
                    qs = slice(512 * qh, 512 * (qh + 1))
                    Op = [opl.tile([128, 512], F32, name=f"Op{j}", tag=f"Op{j}")
                          for j in range(2)]

                    def emit_zero():
                        # hardware start=True only zeroes the region an MM
                        # writes; partial-width accumulation chains must land
                        # on explicitly zeroed banks (stale PSUM junk leaks
                        # in otherwise)
                        for j in range(2):
                            nc.tensor.matmul(Op[j][:], lhsT=Z128[:],
                                             rhs=ONES512[:], start=True,
                                             stop=True, skip_group_check=True)
                    emit_zero()
                    # software-pipelined: emit the score quad for m+1 before
                    # the AV quad for m, so the PE never waits on exp results
                    # (sustained dense PE bursts also keep the HAM at 2.4GHz)
                    pts_by_m = {}

                    def emit_s(m):
                        ms = slice(128 * m, 128 * (m + 1))
                        sts = []
                        for g in range(2):
                            st2 = sp.tile([128, 1024], F32, name="st2",
                                          tag="st2")
                            sts.append(st2)
                        for g in range(2):
                            for j in range(2):
                                a = 2 * g + j
                                hh = slice(32 * a, 32 * (a + 1))
                                nc.tensor.matmul(
                                    sts[g][:, 512 * j:512 * (j + 1)],
                                    lhsT=Ksb[hg][hh, ms], rhs=Qsb[hg][hh, qs],
                                    start=True, stop=True,
                                    tile_position=(32 * a, 0))
                        pts = []
                        for g in range(2):
                            pt2 = pb.tile([128, 1024], BF16, name="pt2",
                                          tag="pt2")
                            pts.append(pt2)
                            k = 2 * m + g
                            on_scalar = (k * ACT_UNITS_PER_32) % 32 \
                                < ACT_UNITS_PER_32
                            if on_scalar:
                                nc.scalar.activation(pt2[:], sts[g][:], Exp,
                                                     scale=SCALE)
                            else:
                                nc.vector.tensor_scalar(
                                    pt2.bitcast(mybir.dt.int16)[:], sts[g][:],
                                    SCH_A, SCH_B,
                                    mybir.AluOpType.mult, mybir.AluOpType.add)
                        pts_by_m[m] = pts

                    def emit_av(m):
                        pts = pts_by_m.pop(m)
                        last = m == 31
                        # pair the two col-groups per j so they run
                        # concurrently, and finish pts[0] consumers first
                        for j, b in ((0, 0), (0, 1), (1, 0), (1, 1)):
                            a = 2 * j + b
                            H = 4 * hg + a
                            vs = slice(33 * H, 33 * H + 33)
                            ps = slice(512 * b, 512 * (b + 1))
                            nc.tensor.matmul(
                                Op[j][64 * b:64 * b + 33, :],
                                lhsT=Vsb[m][:, vs], rhs=pts[j][:, ps],
                                start=False, stop=last,
                                tile_position=(0, 64 * b),
                                skip_group_check=True)

                    emit_s(0)
                    for m in range(32):
                        if m + 1 < 32:
                            emit_s(m + 1)
                        emit_av(m)
                        if inject is not None and m == inject_m:
                            inject()
                    # denominators sit at rows 32 (head A) and 96 (head B) of
                    # each pair bank; copy out, PE-broadcast into the spare
                    # rows 32-63 / 96-127, reciprocal, then normalize into Osb
                    for j in range(2):
                        d4s = []
                        for b in range(2):
                            d4 = db.tile([1, 512], F32, name=f"d4_{j}{b}",
                                         tag=f"d4_{j}{b}", bufs=2)
                            nc.vector.tensor_copy(
                                d4[:], Op[j][64 * b + 32:64 * b + 33, :])
                            d4s.append(d4)
                        for b in range(2):
                            nc.tensor.matmul(
                                Op[j][64 * b + 32:64 * b + 64, :],
                                lhsT=ONES32F[:],
                                rhs=d4s[b][:], start=True, stop=True,
                                tile_position=(0, 64 * b + 32),
                                skip_group_check=True)
                        rj = rb.tile([128, 512], F32, name=f"rj{j}",
                                     tag=f"rj{j}", bufs=2)
                        nc.vector.reciprocal_approx_fast(
                            out=rj[:], in_=Op[j][:, :])
                        for b in range(2):
                            a = 2 * j + b
                            nc.vector.tensor_tensor(
                                Osb[hg][32 * a:32 * a + 32, qs],
                                Op[j][64 * b:64 * b + 32, :],
                                rj[64 * b + 32:64 * b + 64, :],
                                mybir.AluOpType.mult)

            run_phase(0, 0)
            run_phase(1, 0)
            # qh=0 output projection injected into the next phase's m-loop
            # (by m=3 the qh=0 normalize chain has drained; the PE keeps
            # streaming scores in the meantime)
            run_phase(0, 1, inject=lambda: emit_oproj(0))
            run_phase(1, 1)
            emit_oproj(1)


_NC = None


def _get_program():
    global _NC
    if _NC is None:
        _NC = _build_program()
    return _NC


def kernel(x, qkv_w, proj_w, gamma, _trace=False):
    """Full inputs in, full output out. Shards across 8 NeuronCores internally."""
    nc = _get_program()
    B = x.shape[0]
    xf = np.ascontiguousarray(x.reshape(B, C, N).astype(np.float32))
    xf_bf = xf.astype(NPBF16)

    wqT = np.ascontiguousarray(qkv_w[0:256].T.astype(NPBF16))
    wkT = np.ascontiguousarray(qkv_w[256:512].T.astype(NPBF16))
    wvT = np.ascontiguousarray(qkv_w[512:768].T.astype(NPBF16))
    pjT = np.ascontiguousarray(proj_w.T.astype(NPBF16))
    gam = np.full((128, 1), np.float32(gamma.reshape(-1)[0]), dtype=np.float32)

    in_maps = []
    for core in range(8):
        b, qi = divmod(core, 4)
        qs = slice(Q * qi, Q * (qi + 1))
        in_maps.append({
            "xb": xf_bf[b],
            "xqb": np.ascontiguousarray(xf_bf[b][:, qs]),
            "xq": np.ascontiguousarray(xf[b][:, qs]),
            "wkT": wkT, "wqT": wqT, "wvT": wvT, "pjT": pjT,
            "gam": gam,
        })

    res = run_bass_kernel_spmd(nc, in_maps, core_ids=list(range(8)), trace=_trace)

    outf = np.empty((B, C, N), dtype=np.float32)
    for core in range(8):
        b, qi = divmod(core, 4)
        outf[b][:, Q * qi:Q * (qi + 1)] = res.results[core]["out"]
    result = outf.reshape(x.shape)
    if _trace:
        return result, res
    return result



# revision 21
# speedup vs baseline: 1.3935x; 1.0069x over previous
"""Multi-head self-attention 2d (B=2, C=256, H=W=64, 8 heads x 32 dim) on 8 TRN2 cores.

Sharding: batch (2-way) x query-rows-of-N=H*W (4-way) => 8 cores, no collectives.
Each core computes, for its (batch b, query shard of 1024 rows):
  - K', V projections over the FULL 4096 positions of its batch (4x replicated work,
    but removes all cross-core communication),
  - Q' projection for its 1024 query rows,
  - S^T = K'^T-layout scores via tensor-engine row-packing (contraction d=32, 4 heads
    concurrently in the 128x128 PE array),
  - softmax without max-subtraction (scores are ~N(0,1); exp on ScalarE, denominator
    via ones-matmul, division on VectorE),
  - attn @ V via col-packed matmuls accumulating over the 4096 positions,
  - output projection, then out = gamma * proj + x on its [256, 1024] slice.
"""

import os
import sys

import numpy as np

for _p in ("/opt/trn_rl_repo", "/root/.axon_site/_ro/trn_rl_repo"):
    if os.path.isdir(_p) and _p not in sys.path:
        sys.path.insert(0, _p)

import ml_dtypes
import concourse.bacc as bacc
import concourse.bass as bass
import concourse.tile as tile
from concourse import mybir
from concourse.bass_utils import run_bass_kernel_spmd

BF16 = mybir.dt.bfloat16
F32 = mybir.dt.float32
NPBF16 = ml_dtypes.bfloat16

NH, D = 8, 32          # heads, head dim
C = 256                # channels
N = 4096               # H*W positions
Q = 1024               # query shard per core
SCALE = 1.0 / np.sqrt(D)

# Schraudolph bf16 exp on the vector engine: bf16_bits(exp(y)) ~= round(y*128/ln2
# + (127*128 - c)). Fold the attention scale into the multiplier. c tuned for
# min max-rel-error under round-to-nearest (~3.3%).
SCH_A = float(SCALE * 128.0 / np.log(2.0))
SCH_B = float(127.0 * 128.0 - 5.5)
# Fraction of exp tiles computed on ScalarE (rest on VectorE via Schraudolph),
# chosen to balance the two engines' total busy time (measured: ACT 1115ns,
# TS 1222ns per [128,1024] tile, plus ~90ns/m of other VectorE work).
ACT_UNITS_PER_32 = 18


def _build_program():
    nc = bacc.Bacc("TRN2", target_bir_lowering=False, debug=False)

    xb = nc.dram_tensor("xb", [C, N], BF16, kind="ExternalInput")
    xqb = nc.dram_tensor("xqb", [C, Q], BF16, kind="ExternalInput")
    xq = nc.dram_tensor("xq", [C, Q], F32, kind="ExternalInput")
    wkT = nc.dram_tensor("wkT", [C, C], BF16, kind="ExternalInput")
    wqT = nc.dram_tensor("wqT", [C, C], BF16, kind="ExternalInput")
    wvT = nc.dram_tensor("wvT", [C, C], BF16, kind="ExternalInput")
    pjT = nc.dram_tensor("pjT", [C, C], BF16, kind="ExternalInput")
    gam = nc.dram_tensor("gam", [128, 1], F32, kind="ExternalInput")
    out = nc.dram_tensor("out", [C, Q], F32, kind="ExternalOutput")

    with tile.TileContext(nc) as tc:
        _emit(tc, xb, xqb, xq, wkT, wqT, wvT, pjT, gam, out)
    nc.compile()
    return nc


def _emit(tc, xb, xqb, xq, wkT, wqT, wvT, pjT, gam, out):
    from contextlib import ExitStack

    nc = tc.nc
    Exp = mybir.ActivationFunctionType.Exp

    with ExitStack() as ctx:
        per = ctx.enter_context(tc.tile_pool(name="persist", bufs=1))

        def ptile(name, shape, dtype):
            return per.tile(shape, dtype, name=name, tag=name)

        XB = [ptile(f"XB{i}", [128, N], BF16) for i in range(2)]
        XQB = [ptile(f"XQB{i}", [128, Q], BF16) for i in range(2)]
        XQ = [ptile(f"XQ{i}", [128, Q], F32) for i in range(2)]
        WK = [ptile(f"WK{i}", [128, C], BF16) for i in range(2)]
        WQ = [ptile(f"WQ{i}", [128, C], BF16) for i in range(2)]
        WV = [ptile(f"WV{i}", [128, C], BF16) for i in range(2)]
        PJ = [ptile(f"PJ{i}", [128, C], BF16) for i in range(2)]
        G = ptile("G", [128, 1], F32)
        ONE = ptile("ONE", [128, 1], BF16)
        Z128 = ptile("Z128", [1, 128], BF16)   # zeros, lhsT of bank-zeroing matmul
        ONES512 = ptile("ONES512", [1, 512], BF16)
        ONES32F = ptile("ONES32F", [1, 32], F32)  # lhsT of denominator-broadcast matmul
        Ksb = [ptile(f"Ksb{i}", [128, N], BF16) for i in range(2)]
        Qsb = [ptile(f"Qsb{i}", [128, Q], BF16) for i in range(2)]
        Vsb = [ptile(f"Vsb{m}", [128, NH * 33], BF16) for m in range(32)]
        Osb = [ptile(f"Osb{i}", [128, Q], BF16) for i in range(2)]

        # small critical tensors first so Q'/K' projections start ASAP
        for i in range(2):
            r = slice(128 * i, 128 * (i + 1))
            nc.sync.dma_start(WQ[i][:], wqT[r, :])
            nc.sync.dma_start(XQB[i][:], xqb[r, :])
            nc.sync.dma_start(WK[i][:], wkT[r, :])
            nc.sync.dma_start(WV[i][:], wvT[r, :])
        for ch in range(4):  # chunked so projections can start early
            cs_ = slice(1024 * ch, 1024 * (ch + 1))
            for i in range(2):
                r = slice(128 * i, 128 * (i + 1))
                nc.sync.dma_start(XB[i][:, cs_], xb[r, cs_])
        for i in range(2):  # proj weights only needed ~150us in
            r = slice(128 * i, 128 * (i + 1))
            nc.sync.dma_start(PJ[i][:], pjT[r, :])
        nc.sync.dma_start(G[:], gam[:, :])
        for i in range(2):
            r = slice(128 * i, 128 * (i + 1))
            nc.sync.dma_start(XQ[i][:], xq[r, :])  # only needed at the end
        nc.vector.memset(ONE[:], 1.0)
        nc.vector.memset(Z128[:], 0.0)
        nc.vector.memset(ONES512[:], 1.0)
        nc.vector.memset(ONES32F[:], 1.0)

        # ---- projections: Q'[hd, q], K'[hd, m], V[m, hd] --------------------
        # PSUM->SBUF copies alternate ScalarE/VectorE so neither engine gates
        # the projection phase on its own.
        with tc.tile_pool(name="pp", bufs=4, space="PSUM") as pp:
            ncopy = 0

            def pcopy(dst, src):
                nonlocal ncopy
                if ncopy % 2 == 0:
                    nc.vector.tensor_copy(dst, src)
                else:
                    nc.scalar.copy(dst, src)
                ncopy += 1

            for p in range(2):
                hs = slice(128 * p, 128 * (p + 1))
                for t in range(2):
                    qp = pp.tile([128, 512], F32, name="qp", tag="pp")
                    ts_ = slice(512 * t, 512 * (t + 1))
                    nc.tensor.matmul(qp[:], lhsT=WQ[0][:, hs], rhs=XQB[0][:, ts_],
                                     start=True, stop=False)
                    nc.tensor.matmul(qp[:], lhsT=WQ[1][:, hs], rhs=XQB[1][:, ts_],
                                     start=False, stop=True)
                    pcopy(Qsb[p][:, ts_], qp[:])
            for t in range(8):
                ts_ = slice(512 * t, 512 * (t + 1))
                for p in range(2):
                    hs = slice(128 * p, 128 * (p + 1))
                    kp = pp.tile([128, 512], F32, name="kp", tag="pp")
                    nc.tensor.matmul(kp[:], lhsT=WK[0][:, hs], rhs=XB[0][:, ts_],
                                     start=True, stop=False)
                    nc.tensor.matmul(kp[:], lhsT=WK[1][:, hs], rhs=XB[1][:, ts_],
                                     start=False, stop=True)
                    pcopy(Ksb[p][:, ts_], kp[:])
            for m in range(32):
                vp = pp.tile([128, 512], F32, name="vp", tag="pp")
                ms = slice(128 * m, 128 * (m + 1))
                nc.tensor.matmul(vp[:, :C], lhsT=XB[0][:, ms], rhs=WV[0][:],
                                 start=True, stop=False)
                nc.tensor.matmul(vp[:, :C], lhsT=XB[1][:, ms], rhs=WV[1][:],
                                 start=False, stop=True)
                v3 = Vsb[m].rearrange("p (h w) -> p h w", w=33)
                pcopy(v3[:, :, 0:32],
                      vp[:, :C].rearrange("p (h w) -> p h w", w=32))
                nc.vector.memset(v3[:, :, 32:33], 1.0)

        # ---- attention ------------------------------------------------------
        with ExitStack() as actx:
            sp = actx.enter_context(tc.tile_pool(name="sp", bufs=3, space="PSUM"))
            opl = actx.enter_context(tc.tile_pool(name="opl", bufs=1, space="PSUM"))
            pb = actx.enter_context(tc.tile_pool(name="pb", bufs=4))
            db = actx.enter_context(tc.tile_pool(name="db", bufs=8))
            rb = actx.enter_context(tc.tile_pool(name="rb", bufs=2))
            ob = actx.enter_context(tc.tile_pool(name="ob", bufs=4))

            def emit_oproj(qh):
                # output projection + residual for one q-half; borrows an sp
                # score slot for PSUM so it can overlap a running phase
                qs = slice(512 * qh, 512 * (qh + 1))
                pjp = sp.tile([128, 1024], F32, name="pjp", tag="st2")
                for ct in range(2):
                    cs = slice(128 * ct, 128 * (ct + 1))
                    pp2 = pjp[:, 512 * ct:512 * (ct + 1)]
                    nc.tensor.matmul(pp2, lhsT=PJ[0][:, cs], rhs=Osb[0][:, qs],
                                     start=True, stop=False,
                                     skip_group_check=True)
                    nc.tensor.matmul(pp2, lhsT=PJ[1][:, cs], rhs=Osb[1][:, qs],
                                     start=False, stop=True,
                                     skip_group_check=True)
                    obt = ob.tile([128, 512], F32, name="obt", tag="obt")
                    nc.vector.scalar_tensor_tensor(
                        obt[:], pp2, G[:], XQ[ct][:, qs],
                        mybir.AluOpType.mult, mybir.AluOpType.add)
                    nc.sync.dma_start(out[cs, qs], obt[:])

            # qh outer so the qh=0 output projection can overlap the qh=1
            # phases; hg inner.
            def run_phase(hg, qh, prev_tail=None, inject=None, inject_m=3):
                    qs = slice(512 * qh, 512 * (qh + 1))
                    Op = [opl.tile([128, 512], F32, name=f"Op{j}", tag=f"Op{j}")
                          for j in range(2)]

                    def emit_zero():
                        # hardware start=True only zeroes the region an MM
                        # writes; partial-width accumulation chains must land
                        # on explicitly zeroed banks (stale PSUM junk leaks
                        # in otherwise)
                        for j in range(2):
                            nc.tensor.matmul(Op[j][:], lhsT=Z128[:],
                                             rhs=ONES512[:], start=True,
                                             stop=True, skip_group_check=True)
                    emit_zero()
                    # software-pipelined: emit the score quad for m+1 before
                    # the AV quad for m, so the PE never waits on exp results
                    # (sustained dense PE bursts also keep the HAM at 2.4GHz)
                    pts_by_m = {}

                    def emit_s(m):
                        ms = slice(128 * m, 128 * (m + 1))
                        sts = []
                        for g in range(2):
                            st2 = sp.tile([128, 1024], F32, name="st2",
                                          tag="st2")
                            sts.append(st2)
                        for g in range(2):
                            for j in range(2):
                                a = 2 * g + j
                                hh = slice(32 * a, 32 * (a + 1))
                                nc.tensor.matmul(
                                    sts[g][:, 512 * j:512 * (j + 1)],
                                    lhsT=Ksb[hg][hh, ms], rhs=Qsb[hg][hh, qs],
                                    start=True, stop=True,
                                    tile_position=(32 * a, 0))
                        pts = []
                        for g in range(2):
                            pt2 = pb.tile([128, 1024], BF16, name="pt2",
                                          tag="pt2")
                            pts.append(pt2)
                            k = 2 * m + g
                            on_scalar = (k * ACT_UNITS_PER_32) % 32 \
                                < ACT_UNITS_PER_32
                            if on_scalar:
                                nc.scalar.activation(pt2[:], sts[g][:], Exp,
                                                     scale=SCALE)
                            else:
                                nc.vector.tensor_scalar(
                                    pt2.bitcast(mybir.dt.int16)[:], sts[g][:],
                                    SCH_A, SCH_B,
                                    mybir.AluOpType.mult, mybir.AluOpType.add)
                        pts_by_m[m] = pts

                    def emit_av(m):
                        pts = pts_by_m.pop(m)
                        last = m == 31
                        # pair the two col-groups per j so they run
                        # concurrently, and finish pts[0] consumers first
                        for j, b in ((0, 0), (0, 1), (1, 0), (1, 1)):
                            a = 2 * j + b
                            H = 4 * hg + a
                            vs = slice(33 * H, 33 * H + 33)
                            ps = slice(512 * b, 512 * (b + 1))
                            nc.tensor.matmul(
                                Op[j][64 * b:64 * b + 33, :],
                                lhsT=Vsb[m][:, vs], rhs=pts[j][:, ps],
                                start=False, stop=last,
                                tile_position=(0, 64 * b),
                                skip_group_check=True)

                    emit_s(0)
                    for m in range(32):
                        if m + 1 < 32:
                            emit_s(m + 1)
                        emit_av(m)
                        if inject is not None and m == inject_m:
                            inject()
                    # denominators sit at rows 32 (head A) and 96 (head B) of
                    # each pair bank; copy out, PE-broadcast into the spare
                    # rows 32-63 / 96-127, reciprocal, then normalize into Osb
                    for j in range(2):
                        d4s = []
                        for b in range(2):
                            d4 = db.tile([1, 512], F32, name=f"d4_{j}{b}",
                                         tag=f"d4_{j}{b}", bufs=2)
                            nc.vector.tensor_copy(
                                d4[:], Op[j][64 * b + 32:64 * b + 33, :])
                            d4s.append(d4)
                        for b in range(2):
                            # warm-keeper: harmless zero-write (overwritten by
                            # the den-broadcast below) keeps the PE's HAM
                            # activity window busy through the VectorE chain
                            nc.tensor.matmul(
                                Op[j][64 * b + 32:64 * b + 64, :],
                                lhsT=Z128[:, 0:32], rhs=ONES512[:],
                                start=True, stop=True,
                                tile_position=(0, 64 * b + 32),
                                skip_group_check=True)
                        for b in range(2):
                            nc.tensor.matmul(
                                Op[j][64 * b + 32:64 * b + 64, :],
                                lhsT=ONES32F[:],
                                rhs=d4s[b][:], start=True, stop=True,
                                tile_position=(0, 64 * b + 32),
                                skip_group_check=True)
                        rj = rb.tile([128, 512], F32, name=f"rj{j}",
                                     tag=f"rj{j}", bufs=2)
                        nc.vector.reciprocal_approx_fast(
                            out=rj[:], in_=Op[j][:, :])
                        for b in range(2):
                            a = 2 * j + b
                            nc.vector.tensor_tensor(
                                Osb[hg][32 * a:32 * a + 32, qs],
                                Op[j][64 * b:64 * b + 32, :],
                                rj[64 * b + 32:64 * b + 64, :],
                                mybir.AluOpType.mult)

            run_phase(0, 0)
            run_phase(1, 0)
            # qh=0 output projection injected into the next phase's m-loop
            # (by m=3 the qh=0 normalize chain has drained; the PE keeps
            # streaming scores in the meantime)
            run_phase(0, 1, inject=lambda: emit_oproj(0))
            run_phase(1, 1)
            emit_oproj(1)


_NC = None


def _get_program():
    global _NC
    if _NC is None:
        _NC = _build_program()
    return _NC


def kernel(x, qkv_w, proj_w, gamma, _trace=False):
    """Full inputs in, full output out. Shards across 8 NeuronCores internally."""
    nc = _get_program()
    B = x.shape[0]
    xf = np.ascontiguousarray(x.reshape(B, C, N).astype(np.float32))
    xf_bf = xf.astype(NPBF16)

    wqT = np.ascontiguousarray(qkv_w[0:256].T.astype(NPBF16))
    wkT = np.ascontiguousarray(qkv_w[256:512].T.astype(NPBF16))
    wvT = np.ascontiguousarray(qkv_w[512:768].T.astype(NPBF16))
    pjT = np.ascontiguousarray(proj_w.T.astype(NPBF16))
    gam = np.full((128, 1), np.float32(gamma.reshape(-1)[0]), dtype=np.float32)

    in_maps = []
    for core in range(8):
        b, qi = divmod(core, 4)
        qs = slice(Q * qi, Q * (qi + 1))
        in_maps.append({
            "xb": xf_bf[b],
            "xqb": np.ascontiguousarray(xf_bf[b][:, qs]),
            "xq": np.ascontiguousarray(xf[b][:, qs]),
            "wkT": wkT, "wqT": wqT, "wvT": wvT, "pjT": pjT,
            "gam": gam,
        })

    res = run_bass_kernel_spmd(nc, in_maps, core_ids=list(range(8)), trace=_trace)

    outf = np.empty((B, C, N), dtype=np.float32)
    for core in range(8):
        b, qi = divmod(core, 4)
        outf[b][:, Q * qi:Q * (qi + 1)] = res.results[core]["out"]
    result = outf.reshape(x.shape)
    if _trace:
        return result, res
    return result



# revision 22
# speedup vs baseline: 1.4525x; 1.0423x over previous
"""Multi-head self-attention 2d (B=2, C=256, H=W=64, 8 heads x 32 dim) on 8 TRN2 cores.

Sharding: batch (2-way) x query-rows-of-N=H*W (4-way) => 8 cores, no collectives.
Each core computes, for its (batch b, query shard of 1024 rows):
  - K', V projections over the FULL 4096 positions of its batch (4x replicated work,
    but removes all cross-core communication),
  - Q' projection for its 1024 query rows,
  - S^T = K'^T-layout scores via tensor-engine row-packing (contraction d=32, 4 heads
    concurrently in the 128x128 PE array),
  - softmax without max-subtraction (scores are ~N(0,1); exp on ScalarE, denominator
    via ones-matmul, division on VectorE),
  - attn @ V via col-packed matmuls accumulating over the 4096 positions,
  - output projection, then out = gamma * proj + x on its [256, 1024] slice.
"""

import os
import sys

import numpy as np

for _p in ("/opt/trn_rl_repo", "/root/.axon_site/_ro/trn_rl_repo"):
    if os.path.isdir(_p) and _p not in sys.path:
        sys.path.insert(0, _p)

import ml_dtypes
import concourse.bacc as bacc
import concourse.bass as bass
import concourse.tile as tile
from concourse import mybir
from concourse.bass_utils import run_bass_kernel_spmd

BF16 = mybir.dt.bfloat16
F32 = mybir.dt.float32
NPBF16 = ml_dtypes.bfloat16

NH, D = 8, 32          # heads, head dim
C = 256                # channels
N = 4096               # H*W positions
Q = 1024               # query shard per core
SCALE = 1.0 / np.sqrt(D)

# Schraudolph bf16 exp on the vector engine: bf16_bits(exp(y)) ~= round(y*128/ln2
# + (127*128 - c)). Fold the attention scale into the multiplier. c tuned for
# min max-rel-error under round-to-nearest (~3.3%).
SCH_A = float(SCALE * 128.0 / np.log(2.0))
SCH_B = float(127.0 * 128.0 - 5.5)
# Fraction of exp tiles computed on ScalarE (rest on VectorE via Schraudolph),
# chosen to balance the two engines' total busy time (measured: ACT 1115ns,
# TS 1222ns per [128,1024] tile, plus ~90ns/m of other VectorE work).
ACT_UNITS_PER_32 = 16


def _build_program():
    nc = bacc.Bacc("TRN2", target_bir_lowering=False, debug=False)

    xb = nc.dram_tensor("xb", [C, N], BF16, kind="ExternalInput")
    xqb = nc.dram_tensor("xqb", [C, Q], BF16, kind="ExternalInput")
    xq = nc.dram_tensor("xq", [C, Q], F32, kind="ExternalInput")
    wkT = nc.dram_tensor("wkT", [C, C], BF16, kind="ExternalInput")
    wqT = nc.dram_tensor("wqT", [C, C], BF16, kind="ExternalInput")
    wvT = nc.dram_tensor("wvT", [C, C], BF16, kind="ExternalInput")
    pjT = nc.dram_tensor("pjT", [C, C], BF16, kind="ExternalInput")
    gam = nc.dram_tensor("gam", [128, 1], F32, kind="ExternalInput")
    out = nc.dram_tensor("out", [C, Q], F32, kind="ExternalOutput")

    with tile.TileContext(nc) as tc:
        _emit(tc, xb, xqb, xq, wkT, wqT, wvT, pjT, gam, out)
    nc.compile()
    return nc


def _emit(tc, xb, xqb, xq, wkT, wqT, wvT, pjT, gam, out):
    from contextlib import ExitStack

    nc = tc.nc
    Exp = mybir.ActivationFunctionType.Exp

    with ExitStack() as ctx:
        per = ctx.enter_context(tc.tile_pool(name="persist", bufs=1))

        def ptile(name, shape, dtype):
            return per.tile(shape, dtype, name=name, tag=name)

        XB = [ptile(f"XB{i}", [128, N], BF16) for i in range(2)]
        XQB = [ptile(f"XQB{i}", [128, Q], BF16) for i in range(2)]
        XQ = [ptile(f"XQ{i}", [128, Q], F32) for i in range(2)]
        WK = [ptile(f"WK{i}", [128, C], BF16) for i in range(2)]
        WQ = [ptile(f"WQ{i}", [128, C], BF16) for i in range(2)]
        WV = [ptile(f"WV{i}", [128, C], BF16) for i in range(2)]
        PJ = [ptile(f"PJ{i}", [128, C], BF16) for i in range(2)]
        G = ptile("G", [128, 1], F32)
        ONE = ptile("ONE", [128, 1], BF16)
        Z128 = ptile("Z128", [1, 128], BF16)   # zeros, lhsT of bank-zeroing matmul
        ONES512 = ptile("ONES512", [1, 512], BF16)
        ONES32F = ptile("ONES32F", [1, 32], F32)  # lhsT of denominator-broadcast matmul
        Ksb = [ptile(f"Ksb{i}", [128, N], BF16) for i in range(2)]
        Qsb = [ptile(f"Qsb{i}", [128, Q], BF16) for i in range(2)]
        Vsb = [ptile(f"Vsb{m}", [128, NH * 33], BF16) for m in range(32)]
        Osb = [ptile(f"Osb{i}", [128, Q], BF16) for i in range(2)]

        # small critical tensors first so Q'/K' projections start ASAP
        for i in range(2):
            r = slice(128 * i, 128 * (i + 1))
            nc.sync.dma_start(WQ[i][:], wqT[r, :])
            nc.sync.dma_start(XQB[i][:], xqb[r, :])
            nc.sync.dma_start(WK[i][:], wkT[r, :])
            nc.sync.dma_start(WV[i][:], wvT[r, :])
        for ch in range(4):  # chunked so projections can start early
            cs_ = slice(1024 * ch, 1024 * (ch + 1))
            for i in range(2):
                r = slice(128 * i, 128 * (i + 1))
                nc.sync.dma_start(XB[i][:, cs_], xb[r, cs_])
        for i in range(2):  # proj weights only needed ~150us in
            r = slice(128 * i, 128 * (i + 1))
            nc.sync.dma_start(PJ[i][:], pjT[r, :])
        nc.sync.dma_start(G[:], gam[:, :])
        for i in range(2):
            r = slice(128 * i, 128 * (i + 1))
            nc.sync.dma_start(XQ[i][:], xq[r, :])  # only needed at the end
        nc.vector.memset(ONE[:], 1.0)
        nc.vector.memset(Z128[:], 0.0)
        nc.vector.memset(ONES512[:], 1.0)
        nc.vector.memset(ONES32F[:], 1.0)

        # ---- projections: Q'[hd, q], K'[hd, m], V[m, hd] --------------------
        # PSUM->SBUF copies alternate ScalarE/VectorE so neither engine gates
        # the projection phase on its own.
        with tc.tile_pool(name="pp", bufs=4, space="PSUM") as pp:
            ncopy = 0

            def pcopy(dst, src):
                nonlocal ncopy
                if ncopy % 2 == 0:
                    nc.vector.tensor_copy(dst, src)
                else:
                    nc.scalar.copy(dst, src)
                ncopy += 1

            for p in range(2):
                hs = slice(128 * p, 128 * (p + 1))
                for t in range(2):
                    qp = pp.tile([128, 512], F32, name="qp", tag="pp")
                    ts_ = slice(512 * t, 512 * (t + 1))
                    nc.tensor.matmul(qp[:], lhsT=WQ[0][:, hs], rhs=XQB[0][:, ts_],
                                     start=True, stop=False)
                    nc.tensor.matmul(qp[:], lhsT=WQ[1][:, hs], rhs=XQB[1][:, ts_],
                                     start=False, stop=True)
                    pcopy(Qsb[p][:, ts_], qp[:])
            for t in range(8):
                ts_ = slice(512 * t, 512 * (t + 1))
                for p in range(2):
                    hs = slice(128 * p, 128 * (p + 1))
                    kp = pp.tile([128, 512], F32, name="kp", tag="pp")
                    nc.tensor.matmul(kp[:], lhsT=WK[0][:, hs], rhs=XB[0][:, ts_],
                                     start=True, stop=False)
                    nc.tensor.matmul(kp[:], lhsT=WK[1][:, hs], rhs=XB[1][:, ts_],
                                     start=False, stop=True)
                    pcopy(Ksb[p][:, ts_], kp[:])
            for m in range(32):
                vp = pp.tile([128, 512], F32, name="vp", tag="pp")
                ms = slice(128 * m, 128 * (m + 1))
                nc.tensor.matmul(vp[:, :C], lhsT=XB[0][:, ms], rhs=WV[0][:],
                                 start=True, stop=False)
                nc.tensor.matmul(vp[:, :C], lhsT=XB[1][:, ms], rhs=WV[1][:],
                                 start=False, stop=True)
                v3 = Vsb[m].rearrange("p (h w) -> p h w", w=33)
                pcopy(v3[:, :, 0:32],
                      vp[:, :C].rearrange("p (h w) -> p h w", w=32))
                nc.vector.memset(v3[:, :, 32:33], 1.0)

        # ---- attention ------------------------------------------------------
        with ExitStack() as actx:
            sp = actx.enter_context(tc.tile_pool(name="sp", bufs=3, space="PSUM"))
            opl = actx.enter_context(tc.tile_pool(name="opl", bufs=1, space="PSUM"))
            pb = actx.enter_context(tc.tile_pool(name="pb", bufs=4))
            db = actx.enter_context(tc.tile_pool(name="db", bufs=8))
            rb = actx.enter_context(tc.tile_pool(name="rb", bufs=2))
            ob = actx.enter_context(tc.tile_pool(name="ob", bufs=4))

            def emit_oproj(qh):
                # output projection + residual for one q-half; borrows an sp
                # score slot for PSUM so it can overlap a running phase
                qs = slice(512 * qh, 512 * (qh + 1))
                pjp = sp.tile([128, 1024], F32, name="pjp", tag="st2")
                for ct in range(2):
                    cs = slice(128 * ct, 128 * (ct + 1))
                    pp2 = pjp[:, 512 * ct:512 * (ct + 1)]
                    nc.tensor.matmul(pp2, lhsT=PJ[0][:, cs], rhs=Osb[0][:, qs],
                                     start=True, stop=False,
                                     skip_group_check=True)
                    nc.tensor.matmul(pp2, lhsT=PJ[1][:, cs], rhs=Osb[1][:, qs],
                                     start=False, stop=True,
                                     skip_group_check=True)
                    obt = ob.tile([128, 512], F32, name="obt", tag="obt")
                    nc.vector.scalar_tensor_tensor(
                        obt[:], pp2, G[:], XQ[ct][:, qs],
                        mybir.AluOpType.mult, mybir.AluOpType.add)
                    nc.sync.dma_start(out[cs, qs], obt[:])

            # qh outer so the qh=0 output projection can overlap the qh=1
            # phases; hg inner.
            def run_phase(hg, qh, prev_tail=None, inject=None, inject_m=3):
                    qs = slice(512 * qh, 512 * (qh + 1))
                    Op = [opl.tile([128, 512], F32, name=f"Op{j}", tag=f"Op{j}")
                          for j in range(2)]

                    def emit_zero():
                        # hardware start=True only zeroes the region an MM
                        # writes; partial-width accumulation chains must land
                        # on explicitly zeroed banks (stale PSUM junk leaks
                        # in otherwise)
                        for j in range(2):
                            nc.tensor.matmul(Op[j][:], lhsT=Z128[:],
                                             rhs=ONES512[:], start=True,
                                             stop=True, skip_group_check=True)
                    emit_zero()
                    # software-pipelined: emit the score quad for m+1 before
                    # the AV quad for m, so the PE never waits on exp results
                    # (sustained dense PE bursts also keep the HAM at 2.4GHz)
                    pts_by_m = {}

                    def emit_s(m):
                        ms = slice(128 * m, 128 * (m + 1))
                        sts = []
                        for g in range(2):
                            st2 = sp.tile([128, 1024], F32, name="st2",
                                          tag="st2")
                            sts.append(st2)
                        for g in range(2):
                            for j in range(2):
                                a = 2 * g + j
                                hh = slice(32 * a, 32 * (a + 1))
                                nc.tensor.matmul(
                                    sts[g][:, 512 * j:512 * (j + 1)],
                                    lhsT=Ksb[hg][hh, ms], rhs=Qsb[hg][hh, qs],
                                    start=True, stop=True,
                                    tile_position=(32 * a, 0))
                        pts = []
                        for g in range(2):
                            pt2 = pb.tile([128, 1024], BF16, name="pt2",
                                          tag="pt2")
                            pts.append(pt2)
                            k = 2 * m + g
                            # first 2 m-tiles -> ScalarE: at the phase seam
                            # VectorE must run the previous phase's
                            # denominator/normalize chain with an empty queue
                            if m < 2:
                                on_scalar = True
                            else:
                                on_scalar = (k * ACT_UNITS_PER_32) % 32 \
                                    < ACT_UNITS_PER_32
                            if on_scalar:
                                nc.scalar.activation(pt2[:], sts[g][:], Exp,
                                                     scale=SCALE)
                            else:
                                nc.vector.tensor_scalar(
                                    pt2.bitcast(mybir.dt.int16)[:], sts[g][:],
                                    SCH_A, SCH_B,
                                    mybir.AluOpType.mult, mybir.AluOpType.add)
                        pts_by_m[m] = pts

                    def emit_av(m):
                        pts = pts_by_m.pop(m)
                        last = m == 31
                        # pair the two col-groups per j so they run
                        # concurrently, and finish pts[0] consumers first
                        for j, b in ((0, 0), (0, 1), (1, 0), (1, 1)):
                            a = 2 * j + b
                            H = 4 * hg + a
                            vs = slice(33 * H, 33 * H + 33)
                            ps = slice(512 * b, 512 * (b + 1))
                            nc.tensor.matmul(
                                Op[j][64 * b:64 * b + 33, :],
                                lhsT=Vsb[m][:, vs], rhs=pts[j][:, ps],
                                start=False, stop=last,
                                tile_position=(0, 64 * b),
                                skip_group_check=True)

                    emit_s(0)
                    emit_s(1)
                    # previous phase's tail lands here: behind this phase's
                    # first score quads in the PE queue (which keep the HAM
                    # activity window busy) and at the FRONT of VectorE's
                    # queue (the first exps above went to ScalarE)
                    if prev_tail is not None:
                        prev_tail()
                    emit_zero()
                    for m in range(32):
                        emit_av(m)
                        if m + 2 < 32:
                            emit_s(m + 2)
                        if inject is not None and m == inject_m:
                            inject()

                    def tail():
                      # denominators sit at rows 32 (head A) and 96 (head B)
                      # of each pair bank; copy out, PE-broadcast into the
                      # spare rows 32-63 / 96-127, reciprocal, normalize->Osb
                      for j in range(2):
                        d4s = []
                        for b in range(2):
                            d4 = db.tile([1, 512], F32, name=f"d4_{j}{b}",
                                         tag=f"d4_{j}{b}", bufs=2)
                            nc.vector.tensor_copy(
                                d4[:], Op[j][64 * b + 32:64 * b + 33, :])
                            d4s.append(d4)
                        for b in range(2):
                            # warm-keeper: harmless zero-write (overwritten by
                            # the den-broadcast below) keeps the PE's HAM
                            # activity window busy through the VectorE chain
                            nc.tensor.matmul(
                                Op[j][64 * b + 32:64 * b + 64, :],
                                lhsT=Z128[:, 0:32], rhs=ONES512[:],
                                start=True, stop=True,
                                tile_position=(0, 64 * b + 32),
                                skip_group_check=True)
                        for b in range(2):
                            nc.tensor.matmul(
                                Op[j][64 * b + 32:64 * b + 64, :],
                                lhsT=ONES32F[:],
                                rhs=d4s[b][:], start=True, stop=True,
                                tile_position=(0, 64 * b + 32),
                                skip_group_check=True)
                        rj = rb.tile([128, 512], F32, name=f"rj{j}",
                                     tag=f"rj{j}", bufs=2)
                        nc.vector.reciprocal_approx_fast(
                            out=rj[:], in_=Op[j][:, :])
                        for b in range(2):
                            a = 2 * j + b
                            nc.vector.tensor_tensor(
                                Osb[hg][32 * a:32 * a + 32, qs],
                                Op[j][64 * b:64 * b + 32, :],
                                rj[64 * b + 32:64 * b + 64, :],
                                mybir.AluOpType.mult)

                      return None

                    return tail

            t = run_phase(0, 0)
            t = run_phase(1, 0, prev_tail=t)
            # qh=0 output projection injected into the next phase's m-loop
            # (by m=3 the qh=0 normalize chain has drained; the PE keeps
            # streaming scores in the meantime)
            t = run_phase(0, 1, prev_tail=t, inject=lambda: emit_oproj(0))
            t = run_phase(1, 1, prev_tail=t)
            t()
            emit_oproj(1)


_NC = None


def _get_program():
    global _NC
    if _NC is None:
        _NC = _build_program()
    return _NC


def kernel(x, qkv_w, proj_w, gamma, _trace=False):
    """Full inputs in, full output out. Shards across 8 NeuronCores internally."""
    nc = _get_program()
    B = x.shape[0]
    xf = np.ascontiguousarray(x.reshape(B, C, N).astype(np.float32))
    xf_bf = xf.astype(NPBF16)

    wqT = np.ascontiguousarray(qkv_w[0:256].T.astype(NPBF16))
    wkT = np.ascontiguousarray(qkv_w[256:512].T.astype(NPBF16))
    wvT = np.ascontiguousarray(qkv_w[512:768].T.astype(NPBF16))
    pjT = np.ascontiguousarray(proj_w.T.astype(NPBF16))
    gam = np.full((128, 1), np.float32(gamma.reshape(-1)[0]), dtype=np.float32)

    in_maps = []
    for core in range(8):
        b, qi = divmod(core, 4)
        qs = slice(Q * qi, Q * (qi + 1))
        in_maps.append({
            "xb": xf_bf[b],
            "xqb": np.ascontiguousarray(xf_bf[b][:, qs]),
            "xq": np.ascontiguousarray(xf[b][:, qs]),
            "wkT": wkT, "wqT": wqT, "wvT": wvT, "pjT": pjT,
            "gam": gam,
        })

    res = run_bass_kernel_spmd(nc, in_maps, core_ids=list(range(8)), trace=_trace)

    outf = np.empty((B, C, N), dtype=np.float32)
    for core in range(8):
        b, qi = divmod(core, 4)
        outf[b][:, Q * qi:Q * (qi + 1)] = res.results[core]["out"]
    result = outf.reshape(x.shape)
    if _trace:
        return result, res
    return result

